# revision 23
# baseline (speedup 1.0000x reference)
"""CrossSpectralAttention Trainium2 kernel (bf16, pipelined, v5).

Multi-head attention over 48x48 spatial tokens: B=2, C=256, 8 heads x
head_dim 32, N=2304 tokens. Sharded over 8 NeuronCores as 2 batches x 4
head-groups (2 heads per core). Each core computes its heads' Q/K/V
projections, attention, and a partial output projection (column slice of
Wo); the host sums the 4 partials per batch.

The wall-clock floor is the softmax exp: 2 heads x N^2 = 10.6M elements
through ScalarE at 1 elem/cycle/lane = ~83us busy. Everything else is
arranged to keep ScalarE saturated end-to-end:

- Scores are 3-band row-tiled (q/k replicated 3x on 96 partitions); exp
  runs once per group on the full [128, 1536] PSUM block. The 256-wide
  tail q-piece uses 6-chunk groups -> same 1536-col call density.
  NOTE: EXP ACTIVATE with scale=1.0 is ~20% SLOWER than scale!=1.0 on
  this silicon (measured 1848ns vs 1540ns for identical [128,1536]
  calls), so the softmax 1/sqrt(d) scale stays in the instruction.
- PV is 2-band col-tiled (tile_position=(0,0)/(0,64)): chunk j
  accumulates into PSUM partitions 64*(j%2)+[0,33); the drain is one DVE
  copy (band1 -> SBUF) + one DVE add -> oc bf16.
- Softmax denominators ride as a ones-column in vhat. Reciprocals are
  computed DENSELY ([1,qln] den row -> DRAM -> [128,qln/128] -> 1/x ->
  DRAM -> stride-0 broadcast read); a [1,512] reciprocal would be
  1-lane-bound. All hops ride the gpsimd DMA queue (FIFO-ordered).
- A junk-matmul burst at t~10us warms the PE (HAM K=8) before the real
  projections, and a dummy exp pulls the ACT table load into the DMA
  window.
- V^T projection and the h1/q1/k1 projections stream through the h0
  attention slots; slots at piece boundaries absorb the units whose
  deadlines conflict with in-piece slots.
- Output-projection (Wo) chunks are consumed >= 3 groups into the next
  piece so their normalize chains never head-of-line-block the PE FIFO.
- The final 256-token piece skips the broadcast/normalize chain
  entirely: Wo runs per-head on unnormalized oc and the two per-head
  1/den columns are applied per-PARTITION by scalar_tensor_tensor on
  the Wo output (2 DMA hops instead of 4 on the exposed tail).
"""

import numpy as np
import ml_dtypes

import concourse.bass as bass
import concourse.tile as tile
from concourse import mybir
from concourse.bass_utils import run_bass_kernel_spmd

B = 2
C = 256
N = 2304  # 48*48
NH = 8  # total heads
HPC = 2  # heads per core
HD = 32  # head dim
GD = HPC * HD  # 64 dims per core
NC = 8  # cores
NQB = 512  # query-block size for attention
NCH = N // 128  # 18 m-chunks
SCALE = float(HD) ** -0.5

F32 = mybir.dt.float32
BF16 = mybir.dt.bfloat16
NPBF16 = ml_dtypes.bfloat16

LAST_RESULTS = None  # BassKernelResults of the most recent run (for test.py)
_CACHED_NC = None


def _split_excess_waits(nc, max_waits=1):
    """This walrus build allows a single sync-wait per instruction; move
    excess waits onto same-engine NoOps inserted before the instruction."""
    state = {"uid": 0}

    def fix_block(b):
        i = 0
        insts = b.instructions
        while i < len(insts):
            inst = insts[i]
            for sub in getattr(inst, "blocks", None) or []:
                fix_block(sub)
            si = inst.sync_info
            if si is not None and si.on_wait and len(si.on_wait) > max_waits:
                waits = list(si.on_wait)
                keep, extra = waits[:max_waits], waits[max_waits:]
                inst.sync_info = mybir.SyncInfo(
                    on_wait=keep, on_update=list(si.on_update or [])
                )
                nops = []
                for j in range(0, len(extra), max_waits):
                    nop = mybir.InstNoOp(name=f"WSPLIT-{state['uid']}", ins=[], outs=[])
                    state["uid"] += 1
                    nop.engine = inst.engine
                    nop.sync_info = mybir.SyncInfo(
                        on_wait=extra[j : j + max_waits], on_update=[]
                    )
                    nops.append(nop)
                for k, nop in enumerate(nops):
                    insts.insert(i + k, nop)
                i += len(nops)
            i += 1

    for f in nc.m.functions:
        for b in f.blocks:
            fix_block(b)


def _pieces(total, piece):
    out = []
    o = 0
    while o < total:
        ln = min(piece, total - o)
        out.append((o, ln))
        o += ln
    return out


def build_nc(split=True):
    nc = bass.Bass()

    # wq_t/wk_t carry 3 replicated copies of each head's 32 W^T-columns so
    # the projection matmul writes the 3-band PE layout directly.
    # w_all packs wq|wk|wv column-wise: [0:192] q (h-major), [192:384] k,
    # [384:448] v - one DMA for all three projection weights.
    x_d = nc.dram_tensor("x", [C, N], BF16, kind="ExternalInput")
    wall_d = nc.dram_tensor("w_all", [C, 448], BF16, kind="ExternalInput")
    # b4 packs bq|bk per head column-wise: cols q0,q1,k0,k1 (3-band layout)
    b4_d = nc.dram_tensor("b4", [96, 4], F32, kind="ExternalInput")
    # brep packs bv (x3 replicas for batched V^T drains) | bo row-wise
    brep_d = nc.dram_tensor("brep", [1, 3 * GD + C], F32, kind="ExternalInput")
    wo_d = nc.dram_tensor("wo_t", [GD, C], BF16, kind="ExternalInput")
    out_d = nc.dram_tensor("out_t", [N, C], F32, kind="ExternalOutput")

    qpieces = _pieces(N, NQB)
    LASTQ = qpieces[-1][0]  # 2048

    with tile.TileContext(nc) as tc:
        with (
            tc.tile_pool(name="singles", bufs=1) as singles,
            tc.tile_pool(name="expp", bufs=3) as expp,
            tc.tile_pool(name="invp", bufs=4) as invp,
            tc.tile_pool(name="ibcp", bufs=2) as ibcp,
            tc.tile_pool(name="obp", bufs=2) as obp,
            tc.tile_pool(name="outp", bufs=3) as outp,
            tc.tile_pool(name="dram", bufs=1, space="DRAM") as dramp,
        ):
            # ---- inputs to SBUF; first slab small so projections start early
            x_sb = singles.tile([128, 2, N], BF16)
            xr = x_d.rearrange("(c p) n -> p c n", p=128)
            w_sb = singles.tile([128, 2, 448], BF16)
            nc.gpsimd.dma_start(
                out=w_sb, in_=wall_d.rearrange("(c p) d -> p c d", p=128)
            )
            nc.sync.dma_start(out=x_sb[:, :, :512], in_=xr[:, :, :512])
            nc.sync.dma_start(out=x_sb[:, :, 512:1408], in_=xr[:, :, 512:1408])
            nc.sync.dma_start(out=x_sb[:, :, 1408:], in_=xr[:, :, 1408:])
            b4 = singles.tile([96, 4], F32)
            nc.gpsimd.dma_start(out=b4, in_=b4_d[:, :])
            # Wo^T for both heads (one 64-deep matmul)...
            wo2 = singles.tile([GD, C], BF16)
            nc.scalar.dma_start(out=wo2, in_=wo_d[:, :])
            # ...and per-head at base partition 0 (split-Wo tail path)
            wo2h = singles.tile([HD, HPC, C], BF16)
            nc.scalar.dma_start(
                out=wo2h, in_=wo_d.rearrange("(h d) c -> d h c", h=HPC)
            )
            # bv (3 replicas) | bo replicated across partitions
            brep = singles.tile([128, 3 * GD + C], F32)
            nc.scalar.dma_start(
                out=brep,
                in_=bass.AP(
                    tensor=brep_d, offset=0, ap=[[0, 128], [1, 3 * GD + C]]
                ),
            )
            bv_rep3 = brep[:, : 3 * GD]
            bo_rep = brep[:, 3 * GD :]
            wof = {"q": 0, "k": 192}
            bcol = {("q", 0): 0, ("q", 1): 1, ("k", 0): 2, ("k", 1): 3}

            # dummy exp pulls the ACT table load into the DMA window
            dum = invp.tile([1, 4], F32, tag="dum", name="dum")
            nc.gpsimd.memset(dum, 0.0)
            nc.scalar.activation(
                out=dum, in_=dum, func=mybir.ActivationFunctionType.Exp,
                scale=SCALE,
            )
            # q/k in 3-band replicated layout [96, N] per head
            q_rep = [
                singles.tile([96, N], BF16, name=f"qrep{h}", tag=f"qrep{h}")
                for h in range(HPC)
            ]
            k_rep = [
                singles.tile([96, N], BF16, name=f"krep{h}", tag=f"krep{h}")
                for h in range(HPC)
            ]
            dest = {"q": q_rep, "k": k_rep}

            # vhat[:, j, h, :] = [V_t_h(chunk j) | 1] per head
            vhat = singles.tile([128, NCH, HPC, HD + 1], BF16)
            nc.gpsimd.memset(vhat[:, :, :, HD : HD + 1], 1.0)

            # oc[h] rows 0..31: head h's unnormalized output, row 32: its
            # softmax denominator. on2 rows 32h..32h+31: normalized.
            oc = [
                singles.tile([HD + 1, N], BF16, name=f"oc{h}", tag=f"oc{h}")
                for h in range(HPC)
            ]
            on2 = singles.tile([GD, N], BF16)
            # per-head 1/den for the split-Wo tail (chunk-column layout)
            dinv_last = [
                singles.tile([128, 2], F32, name=f"dinvl{h}", tag=f"dinvl{h}")
                for h in range(HPC)
            ]

            def emit_proj(psum_pool, name, h, off, ln, tag="proj"):
                ps = psum_pool.tile([96, 512], F32, tag=tag, name="ps")
                for c in range(2):
                    nc.tensor.matmul(
                        ps[:, :ln],
                        w_sb[:, c, wof[name] + 96 * h : wof[name] + 96 * h + 96],
                        x_sb[:, c, off : off + ln],
                        start=(c == 0),
                        stop=(c == 1),
                    )
                nc.vector.tensor_scalar(
                    out=dest[name][h][:, off : off + ln],
                    in0=ps[:, :ln],
                    scalar1=b4[:, bcol[(name, h)] : bcol[(name, h)] + 1],
                    scalar2=None,
                    op0=mybir.AluOpType.add,
                )

            def emit_vt(psum_pool, b, tag="proj"):
                # V^T for chunks 3b..3b+2, one PSUM tile + one DVE drain
                tp = psum_pool.tile([128, 3 * GD], F32, tag=tag, name="tp")
                for jj in range(3):
                    j = 3 * b + jj
                    for c in range(2):
                        nc.tensor.matmul(
                            tp[:, GD * jj : GD * (jj + 1)],
                            x_sb[:, c, 128 * j : 128 * (j + 1)],
                            w_sb[:, c, 384:448],
                            start=(c == 0),
                            stop=(c == 1),
                        )
                nc.vector.tensor_tensor(
                    out=vhat[:, 3 * b : 3 * b + 3, :, :HD],
                    in0=tp.rearrange("p (j h d) -> p j h d", j=3, h=HPC),
                    in1=bv_rep3.rearrange("p (j h d) -> p j h d", j=3, h=HPC),
                    op=mybir.AluOpType.add,
                )

            # ---- upfront: PE warm-up burst, k(h0) full, q(h0)p0, V^T 0-2
            with tc.tile_pool(name="proj_psum", bufs=4, space="PSUM") as proj_psum:
                emit_proj(proj_psum, "k", 0, *qpieces[0][:2])
                emit_proj(proj_psum, "q", 0, *qpieces[0][:2])
                for off, ln in qpieces[1:]:
                    emit_proj(proj_psum, "k", 0, off, ln)
                emit_vt(proj_psum, 0)
                emit_vt(proj_psum, 1)

            # remaining projection work, streamed through the h0 attention
            # slots. Vt_b must be emitted before PV(g_b) (block g_b+1); q0
            # piece p+1 lands in piece p's BOUNDARY slot (its deadline
            # conflicts with the in-piece Vt slots); k1/q1 anywhere in h0.
            units = [
                ("vt", 2), ("vt", 3), ("q", 0, 1), ("vt", 4), ("vt", 5),
                ("q", 0, 2), ("k", 1, 0), ("k", 1, 1), ("k", 1, 2), ("k", 1, 3),
                ("q", 0, 3), ("k", 1, 4), ("q", 1, 0), ("q", 1, 1), ("q", 1, 2),
                ("q", 0, 4), ("q", 1, 3), ("q", 1, 4),
            ]

            def emit_unit(u):
                if u[0] == "vt":
                    emit_vt(mixp, u[1], tag="mix")
                else:
                    name, h, p = u
                    emit_proj(mixp, name, h, *qpieces[p][:2], tag="mix")

            # ---- attention + normalize + output projection, pipelined ----
            with (
                tc.tile_pool(name="spsum", bufs=2, space="PSUM") as spsum,
                tc.tile_pool(name="opsum", bufs=1, space="PSUM") as opsum,
                tc.tile_pool(name="mixp", bufs=1, space="PSUM") as mixp,
            ):
                den_dram = dramp.tile([HPC, N], BF16, tag="dend")
                inv_dram = dramp.tile([HPC, N], F32, tag="invd")

                def emit_wo(j, pool):
                    tag = "mix" if pool is mixp else "o"
                    wp = pool.tile([128, 512], F32, tag=tag, name="wp")
                    nc.tensor.matmul(
                        wp[:, :C],
                        on2[:, 128 * j : 128 * (j + 1)],
                        wo2,
                        start=True,
                        stop=True,
                    )
                    ot = outp.tile([128, C], F32, tag="ot")
                    nc.vector.tensor_tensor(
                        out=ot, in0=wp[:, :C], in1=bo_rep, op=mybir.AluOpType.add
                    )
                    nc.sync.dma_start(
                        out=out_d[128 * j : 128 * (j + 1), :], in_=ot
                    )

                def emit_wo_split(j, jj, pool):
                    # unnormalized per-head Wo + per-partition 1/den columns
                    tag = "mix" if pool is mixp else "o"
                    wp = pool.tile([128, 512], F32, tag=tag, name="wp")
                    for h in range(HPC):
                        nc.tensor.matmul(
                            wp[:, C * h : C * h + C],
                            oc[h][:HD, 128 * j : 128 * (j + 1)],
                            wo2h[:, h, :],
                            start=True,
                            stop=True,
                        )
                    t1 = outp.tile([128, C], F32, tag="ot")
                    nc.vector.scalar_tensor_tensor(
                        out=t1,
                        in0=wp[:, :C],
                        scalar=dinv_last[0][:, jj : jj + 1],
                        in1=bo_rep,
                        op0=mybir.AluOpType.mult,
                        op1=mybir.AluOpType.add,
                    )
                    ot = outp.tile([128, C], F32, tag="ot")
                    nc.vector.scalar_tensor_tensor(
                        out=ot,
                        in0=wp[:, C : 2 * C],
                        scalar=dinv_last[1][:, jj : jj + 1],
                        in1=t1,
                        op0=mybir.AluOpType.mult,
                        op1=mybir.AluOpType.add,
                    )
                    nc.sync.dma_start(
                        out=out_d[128 * j : 128 * (j + 1), :], in_=ot
                    )

                def emit_pv(h, chunks, ex, o_ps, qln):
                    # chunk j accumulates into PV band j%2 (PSUM partitions
                    # 64*(j%2)+[0,33)); the two bands' column-group matmuls
                    # run concurrently on the PE.
                    for i, j in enumerate(chunks):
                        bb = j % 2
                        nc.tensor.matmul(
                            o_ps[64 * bb : 64 * bb + 33, :qln],
                            vhat[:, j, h, :],
                            ex[:, qln * i : qln * i + qln],
                            start=(j < 2),
                            stop=(j >= NCH - 2),
                            tile_position=(0, 64 * bb),
                        )

                pend = []  # output-projection chunks ready to emit

                def finish_piece(h, qoff, qln, o_ps):
                    # instruction, so stage band1 in SBUF then add band0.
                    # (row 32 = softmax denominator)
                    ob1 = obp.tile([33, NQB], BF16, tag="ob1")
                    nc.vector.tensor_copy(
                        out=ob1[:, :qln], in_=o_ps[64:97, :qln]
                    )
                    nc.vector.tensor_tensor(
                        out=oc[h][:, qoff : qoff + qln],
                        in0=o_ps[0:33, :qln],
                        in1=ob1[:, :qln],
                        op=mybir.AluOpType.add,
                    )
                    # dense reciprocal via DRAM bounce (gpsimd queue FIFO)
                    nc.gpsimd.dma_start(
                        out=den_dram[h : h + 1, qoff : qoff + qln],
                        in_=oc[h][HD : HD + 1, qoff : qoff + qln],
                    )
                    nd = qln // 128
                    dsrc = den_dram[h : h + 1, qoff : qoff + qln]
                    if qoff == LASTQ:
                        # chunk-column layout feeding the split-Wo tail:
                        # partition p, col jj <- token qoff + 128*jj + p
                        dent_bf = invp.tile(
                            [128, 4], BF16, tag="invb", name="dent_bf"
                        )
                        nc.gpsimd.dma_start(
                            out=dent_bf[:, :nd],
                            in_=bass.AP(
                                tensor=dsrc.tensor,
                                offset=dsrc.offset,
                                ap=[[1, 128], [128, nd]],
                            ),
                        )
                        nc.vector.tensor_copy(
                            out=dinv_last[h][:, :nd], in_=dent_bf[:, :nd]
                        )
                        nc.vector.reciprocal(
                            out=dinv_last[h][:, :nd],
                            in_=dinv_last[h][:, :nd],
                        )
                        return  # no broadcast, no normalize, no pend
                    dent_bf = invp.tile(
                        [128, 4], BF16, tag="invb", name="dent_bf"
                    )
                    nc.gpsimd.dma_start(
                        out=dent_bf[:, :nd],
                        in_=bass.AP(
                            tensor=dsrc.tensor,
                            offset=dsrc.offset,
                            ap=[[nd, 128], [1, nd]],
                        ),
                    )
                    dinv = invp.tile([128, 4], F32, tag="invf", name="dinv")
                    nc.vector.tensor_copy(
                        out=dinv[:, :nd], in_=dent_bf[:, :nd]
                    )
                    nc.vector.reciprocal(out=dinv[:, :nd], in_=dinv[:, :nd])
                    idst = inv_dram[h : h + 1, qoff : qoff + qln]
                    nc.gpsimd.dma_start(
                        out=bass.AP(
                            tensor=idst.tensor,
                            offset=idst.offset,
                            ap=[[nd, 128], [1, nd]],
                        ),
                        in_=dinv[:, :nd],
                    )
                    ibc = ibcp.tile([HD, NQB], F32, tag="ibc")
                    src = inv_dram[h : h + 1, qoff : qoff + qln]
                    bc = bass.AP(
                        tensor=src.tensor,
                        offset=src.offset,
                        ap=[[0, HD]] + [list(d) for d in src.ap[1:]],
                    )
                    nc.gpsimd.dma_start(out=ibc[:, :qln], in_=bc)
                    nc.vector.tensor_tensor(
                        out=on2[HD * h : HD * (h + 1), qoff : qoff + qln],
                        in0=oc[h][:HD, qoff : qoff + qln],
                        in1=ibc[:, :qln],
                        op=mybir.AluOpType.mult,
                    )
                    # piece-boundary slot: up to two more wo during h1
                    if h == 1 and pend:
                        emit_wo(pend.pop(0), mixp)
                    if h == 1 and pend:
                        emit_wo(pend.pop(0), opsum)
                    if h == 1:
                        j0 = qoff // 128
                        pend.extend(range(j0, j0 + qln // 128))

                # The final emit_pv of a piece waits on that piece's last exp;
                # emitted at piece end it head-of-line-blocks the next piece's
                # scores in the PE FIFO (~600ns ACTIVATE gap per boundary).
                # Instead, carry it (plus the whole piece epilogue) into the
                # next piece's first group block.
                carry = None
                for h in range(HPC):
                    for qoff, qln in qpieces:
                        cpg = 6 if qln == 256 else 3
                        groups = [
                            list(range(g, min(g + cpg, NCH)))
                            for g in range(0, NCH, cpg)
                        ]
                        o_ps = opsum.tile([128, NQB], F32, tag="o")
                        ex_prev = None
                        chunks_prev = None
                        for gi, chunks in enumerate(groups):
                            s_tri = spsum.tile([128, 3 * NQB], F32, tag="s")
                            for i, j in enumerate(chunks):
                                # chunks sharing a PSUM bank must share a row
                                # band (serialize); distinct banks may differ.
                                a = (i % 3) if qln == NQB else ((i // 2) % 3)
                                nc.tensor.matmul(
                                    s_tri[:, qln * i : qln * i + qln],
                                    k_rep[h][
                                        32 * a : 32 * a + 32,
                                        128 * j : 128 * j + 128,
                                    ],
                                    q_rep[h][
                                        32 * a : 32 * a + 32, qoff : qoff + qln
                                    ],
                                    start=True,
                                    stop=True,
                                )
                            if gi > 0:
                                emit_pv(h, chunks_prev, ex_prev, o_ps, qln)
                            elif carry is not None:
                                ch, cq, cqln, co, cex, cch = carry
                                emit_pv(ch, cch, cex, co, cqln)
                                finish_piece(ch, cq, cqln, co)
                                carry = None
                            if gi >= 1:
                                if h == 0 and units:
                                    emit_unit(units.pop(0))
                                elif pend and gi >= 3:
                                    # >=3: the chunk's normalize chain (~3us)
                                    # must be done or the Wo matmul would
                                    # head-of-line-block the PE FIFO.
                                    emit_wo(pend.pop(0), mixp)
                            ex = expp.tile([128, 3 * NQB], BF16, tag="ex")
                            ncol = qln * len(chunks)
                            nc.scalar.activation(
                                out=ex[:, :ncol],
                                in_=s_tri[:, :ncol],
                                func=mybir.ActivationFunctionType.Exp,
                                scale=SCALE,
                            )
                            ex_prev = ex
                            chunks_prev = chunks
                        carry = (h, qoff, qln, o_ps, ex_prev, chunks_prev)
                # final piece epilogue + leftover output projections,
                # alternating the two free PSUM pools so the chains pipeline
                ch, cq, cqln, co, cex, cch = carry
                emit_pv(ch, cch, cex, co, cqln)
                finish_piece(ch, cq, cqln, co)
                wi = 0
                while pend:
                    emit_wo(pend.pop(0), (mixp, opsum)[wi % 2])
                    wi += 1
                for jj in range(2):
                    emit_wo_split(LASTQ // 128 + jj, jj, (mixp, opsum)[wi % 2])
                    wi += 1

    if split:
        _split_excess_waits(nc)
    return nc


def kernel(x, Wq, bq, Wk, bk, Wv, bv, Wo, bo):
    global LAST_RESULTS, _CACHED_NC
    x = np.asarray(x, dtype=np.float32)
    Wq = np.asarray(Wq, dtype=np.float32)
    Wk = np.asarray(Wk, dtype=np.float32)
    Wv = np.asarray(Wv, dtype=np.float32)
    Wo = np.asarray(Wo, dtype=np.float32)
    bq = np.asarray(bq, dtype=np.float32)
    bk = np.asarray(bk, dtype=np.float32)
    bv = np.asarray(bv, dtype=np.float32)
    bo = np.asarray(bo, dtype=np.float32)

    def wrep(W, g):
        # [C, 2, 96]: head h cols = W[64g+32h : 64g+32h+32, :].T tiled 3x
        out = np.empty((C, HPC, 96), np.float32)
        for h in range(HPC):
            blk = W[GD * g + HD * h : GD * g + HD * (h + 1), :].T  # [C, 32]
            out[:, h, :] = np.tile(blk, (1, 3))
        return out

    def b3(bvec, g, h):
        return np.tile(bvec[GD * g + HD * h : GD * g + HD * (h + 1)], 3)

    xf = x.reshape(B, C, N)
    in_maps = []
    for core in range(NC):
        b = core // 4
        g = core % 4
        sl = slice(GD * g, GD * (g + 1))
        w_all = np.concatenate(
            [
                wrep(Wq, g).reshape(C, 192),
                wrep(Wk, g).reshape(C, 192),
                Wv[sl, :].T,
            ],
            axis=1,
        )
        b4 = np.stack(
            [b3(bq, g, 0), b3(bq, g, 1), b3(bk, g, 0), b3(bk, g, 1)], axis=1
        )
        # bv 3 replicas (for batched V^T drains) then bo
        brep = np.concatenate(
            [np.tile(bv[sl], 3), bo if g == 0 else np.zeros(C, np.float32)]
        ).reshape(1, 3 * GD + C)
        in_maps.append(
            {
                "x": np.ascontiguousarray(xf[b].astype(NPBF16)),
                "w_all": np.ascontiguousarray(w_all.astype(NPBF16)),
                "b4": np.ascontiguousarray(b4),
                "brep": np.ascontiguousarray(brep),
                "wo_t": np.ascontiguousarray(Wo[:, sl].T.astype(NPBF16)),
            }
        )

    if _CACHED_NC is None:
        _CACHED_NC = build_nc()
    res = run_bass_kernel_spmd(_CACHED_NC, in_maps, core_ids=list(range(NC)))
    LAST_RESULTS = res

    out = np.zeros((B, C, N), dtype=np.float32)
    for core in range(NC):
        out[core // 4] += res.results[core]["out_t"].T
    return out.reshape(B, C, 48, 48)


# revision 24
# speedup vs baseline: 1.0207x; 1.0207x over previous
"""CrossSpectralAttention Trainium2 kernel (bf16, pipelined, v5).

Multi-head attention over 48x48 spatial tokens: B=2, C=256, 8 heads x
head_dim 32, N=2304 tokens. Sharded over 8 NeuronCores as 2 batches x 4
head-groups (2 heads per core). Each core computes its heads' Q/K/V
projections, attention, and a partial output projection (column slice of
Wo); the host sums the 4 partials per batch.

The wall-clock floor is the softmax exp: 2 heads x N^2 = 10.6M elements
through ScalarE at 1 elem/cycle/lane = ~83us busy. Everything else is
arranged to keep ScalarE saturated end-to-end:

- Scores are 3-band row-tiled (q/k replicated 3x on 96 partitions); exp
  runs once per group on the full [128, 1536] PSUM block. The 256-wide
  tail q-piece uses 6-chunk groups -> same 1536-col call density.
  NOTE: EXP ACTIVATE with scale=1.0 is ~20% SLOWER than scale!=1.0 on
  this silicon (measured 1848ns vs 1540ns for identical [128,1536]
  calls), so the softmax 1/sqrt(d) scale stays in the instruction.
- PV is 2-band col-tiled (tile_position=(0,0)/(0,64)): chunk j
  accumulates into PSUM partitions 64*(j%2)+[0,33); the drain is one DVE
  copy (band1 -> SBUF) + one DVE add -> oc bf16.
- Softmax denominators ride as a ones-column in vhat. Reciprocals are
  computed DENSELY ([1,qln] den row -> DRAM -> [128,qln/128] -> 1/x ->
  DRAM -> stride-0 broadcast read); a [1,512] reciprocal would be
  1-lane-bound. All hops ride the gpsimd DMA queue (FIFO-ordered).
- A junk-matmul burst at t~10us warms the PE (HAM K=8) before the real
  projections, and a dummy exp pulls the ACT table load into the DMA
  window.
- V^T projection and the h1/q1/k1 projections stream through the h0
  attention slots; slots at piece boundaries absorb the units whose
  deadlines conflict with in-piece slots.
- Output-projection (Wo) chunks are consumed >= 3 groups into the next
  piece so their normalize chains never head-of-line-block the PE FIFO.
- The final 256-token piece skips the broadcast/normalize chain
  entirely: Wo runs per-head on unnormalized oc and the two per-head
  1/den columns are applied per-PARTITION by scalar_tensor_tensor on
  the Wo output (2 DMA hops instead of 4 on the exposed tail).
"""

import numpy as np
import ml_dtypes

import concourse.bass as bass
import concourse.tile as tile
from concourse import mybir
from concourse.bass_utils import run_bass_kernel_spmd

B = 2
C = 256
N = 2304  # 48*48
NH = 8  # total heads
HPC = 2  # heads per core
HD = 32  # head dim
GD = HPC * HD  # 64 dims per core
NC = 8  # cores
NQB = 512  # query-block size for attention
NCH = N // 128  # 18 m-chunks
SCALE = float(HD) ** -0.5

F32 = mybir.dt.float32
BF16 = mybir.dt.bfloat16
NPBF16 = ml_dtypes.bfloat16

LAST_RESULTS = None  # BassKernelResults of the most recent run (for test.py)
_CACHED_NC = None


def _split_excess_waits(nc, max_waits=1):
    """This walrus build allows a single sync-wait per instruction; move
    excess waits onto same-engine NoOps inserted before the instruction."""
    state = {"uid": 0}

    def fix_block(b):
        i = 0
        insts = b.instructions
        while i < len(insts):
            inst = insts[i]
            for sub in getattr(inst, "blocks", None) or []:
                fix_block(sub)
            si = inst.sync_info
            if si is not None and si.on_wait and len(si.on_wait) > max_waits:
                waits = list(si.on_wait)
                keep, extra = waits[:max_waits], waits[max_waits:]
                inst.sync_info = mybir.SyncInfo(
                    on_wait=keep, on_update=list(si.on_update or [])
                )
                nops = []
                for j in range(0, len(extra), max_waits):
                    nop = mybir.InstNoOp(name=f"WSPLIT-{state['uid']}", ins=[], outs=[])
                    state["uid"] += 1
                    nop.engine = inst.engine
                    nop.sync_info = mybir.SyncInfo(
                        on_wait=extra[j : j + max_waits], on_update=[]
                    )
                    nops.append(nop)
                for k, nop in enumerate(nops):
                    insts.insert(i + k, nop)
                i += len(nops)
            i += 1

    for f in nc.m.functions:
        for b in f.blocks:
            fix_block(b)


def _pieces(total, piece):
    out = []
    o = 0
    while o < total:
        ln = min(piece, total - o)
        out.append((o, ln))
        o += ln
    return out


def build_nc(split=True):
    nc = bass.Bass()

    # wq_t/wk_t carry 3 replicated copies of each head's 32 W^T-columns so
    # the projection matmul writes the 3-band PE layout directly.
    # w_all packs wq|wk|wv column-wise: [0:192] q (h-major), [192:384] k,
    # [384:448] v - one DMA for all three projection weights.
    x_d = nc.dram_tensor("x", [C, N], BF16, kind="ExternalInput")
    wall_d = nc.dram_tensor("w_all", [C, 448], BF16, kind="ExternalInput")
    # b4 packs bq|bk per head column-wise: cols q0,q1,k0,k1 (3-band layout)
    b4_d = nc.dram_tensor("b4", [96, 4], F32, kind="ExternalInput")
    # brep packs bv (x3 replicas for batched V^T drains) | bo row-wise
    brep_d = nc.dram_tensor("brep", [1, 3 * GD + C], F32, kind="ExternalInput")
    wo_d = nc.dram_tensor("wo_t", [GD, C], BF16, kind="ExternalInput")
    out_d = nc.dram_tensor("out_t", [N, C], F32, kind="ExternalOutput")

    qpieces = _pieces(N, NQB)
    LASTQ = qpieces[-1][0]  # 2048

    with tile.TileContext(nc) as tc:
        with (
            tc.tile_pool(name="singles", bufs=1) as singles,
            tc.tile_pool(name="expp", bufs=3) as expp,
            tc.tile_pool(name="invp", bufs=4) as invp,
            tc.tile_pool(name="ibcp", bufs=2) as ibcp,
            tc.tile_pool(name="obp", bufs=2) as obp,
            tc.tile_pool(name="outp", bufs=3) as outp,
            tc.tile_pool(name="dram", bufs=1, space="DRAM") as dramp,
        ):
            # ---- inputs to SBUF; first slab small so projections start early
            x_sb = singles.tile([128, 2, N], BF16)
            xr = x_d.rearrange("(c p) n -> p c n", p=128)
            w_sb = singles.tile([128, 2, 448], BF16)
            nc.gpsimd.dma_start(
                out=w_sb, in_=wall_d.rearrange("(c p) d -> p c d", p=128)
            )
            nc.sync.dma_start(out=x_sb[:, :, :512], in_=xr[:, :, :512])
            nc.sync.dma_start(out=x_sb[:, :, 512:1408], in_=xr[:, :, 512:1408])
            nc.sync.dma_start(out=x_sb[:, :, 1408:], in_=xr[:, :, 1408:])
            b4 = singles.tile([96, 4], F32)
            nc.gpsimd.dma_start(out=b4, in_=b4_d[:, :])
            # Wo^T for both heads (one 64-deep matmul)...
            wo2 = singles.tile([GD, C], BF16)
            nc.scalar.dma_start(out=wo2, in_=wo_d[:, :])
            # ...and per-head at base partition 0 (split-Wo tail path)
            wo2h = singles.tile([HD, HPC, C], BF16)
            nc.scalar.dma_start(
                out=wo2h, in_=wo_d.rearrange("(h d) c -> d h c", h=HPC)
            )
            # bv (3 replicas) | bo replicated across partitions
            brep = singles.tile([128, 3 * GD + C], F32)
            nc.scalar.dma_start(
                out=brep,
                in_=bass.AP(
                    tensor=brep_d, offset=0, ap=[[0, 128], [1, 3 * GD + C]]
                ),
            )
            bv_rep3 = brep[:, : 3 * GD]
            bo_rep = brep[:, 3 * GD :]
            wof = {"q": 0, "k": 192}
            bcol = {("q", 0): 0, ("q", 1): 1, ("k", 0): 2, ("k", 1): 3}

            # dummy exp pulls the ACT table load into the DMA window
            dum = invp.tile([1, 4], F32, tag="dum", name="dum")
            nc.gpsimd.memset(dum, 0.0)
            nc.scalar.activation(
                out=dum, in_=dum, func=mybir.ActivationFunctionType.Exp,
                scale=SCALE,
            )
            # q/k in 3-band replicated layout [96, N] per head
            q_rep = [
                singles.tile([96, N], BF16, name=f"qrep{h}", tag=f"qrep{h}")
                for h in range(HPC)
            ]
            k_rep = [
                singles.tile([96, N], BF16, name=f"krep{h}", tag=f"krep{h}")
                for h in range(HPC)
            ]
            dest = {"q": q_rep, "k": k_rep}

            # vhat[:, j, h, :] = [V_t_h(chunk j) | 1] per head
            vhat = singles.tile([128, NCH, HPC, HD + 1], BF16)
            nc.gpsimd.memset(vhat[:, :, :, HD : HD + 1], 1.0)

            # oc[h] rows 0..31: head h's unnormalized output, row 32: its
            # softmax denominator. on2 rows 32h..32h+31: normalized.
            oc = [
                singles.tile([HD + 1, N], BF16, name=f"oc{h}", tag=f"oc{h}")
                for h in range(HPC)
            ]
            on2 = singles.tile([GD, N], BF16)
            # per-head 1/den for the split-Wo tail (chunk-column layout)
            dinv_last = [
                singles.tile([128, 2], F32, name=f"dinvl{h}", tag=f"dinvl{h}")
                for h in range(HPC)
            ]

            def emit_proj(psum_pool, name, h, off, ln, tag="proj"):
                ps = psum_pool.tile([96, 512], F32, tag=tag, name="ps")
                for c in range(2):
                    nc.tensor.matmul(
                        ps[:, :ln],
                        w_sb[:, c, wof[name] + 96 * h : wof[name] + 96 * h + 96],
                        x_sb[:, c, off : off + ln],
                        start=(c == 0),
                        stop=(c == 1),
                    )
                nc.vector.tensor_scalar(
                    out=dest[name][h][:, off : off + ln],
                    in0=ps[:, :ln],
                    scalar1=b4[:, bcol[(name, h)] : bcol[(name, h)] + 1],
                    scalar2=None,
                    op0=mybir.AluOpType.add,
                )

            def emit_vt(psum_pool, b, tag="proj"):
                # V^T for chunks 3b..3b+2, one PSUM tile + one DVE drain
                tp = psum_pool.tile([128, 3 * GD], F32, tag=tag, name="tp")
                for jj in range(3):
                    j = 3 * b + jj
                    for c in range(2):
                        nc.tensor.matmul(
                            tp[:, GD * jj : GD * (jj + 1)],
                            x_sb[:, c, 128 * j : 128 * (j + 1)],
                            w_sb[:, c, 384:448],
                            start=(c == 0),
                            stop=(c == 1),
                        )
                nc.vector.tensor_tensor(
                    out=vhat[:, 3 * b : 3 * b + 3, :, :HD],
                    in0=tp.rearrange("p (j h d) -> p j h d", j=3, h=HPC),
                    in1=bv_rep3.rearrange("p (j h d) -> p j h d", j=3, h=HPC),
                    op=mybir.AluOpType.add,
                )

            # ---- upfront: PE warm-up burst, k(h0) full, q(h0)p0, V^T 0-2
            with tc.tile_pool(name="proj_psum", bufs=4, space="PSUM") as proj_psum:
                emit_proj(proj_psum, "k", 0, *qpieces[0][:2])
                emit_proj(proj_psum, "q", 0, *qpieces[0][:2])
                for off, ln in qpieces[1:]:
                    emit_proj(proj_psum, "k", 0, off, ln)
                emit_vt(proj_psum, 0)
                emit_vt(proj_psum, 1)

            # remaining projection work, streamed through the h0 attention
            # slots. Vt_b must be emitted before PV(g_b) (block g_b+1); q0
            # piece p+1 lands in piece p's BOUNDARY slot (its deadline
            # conflicts with the in-piece Vt slots); k1/q1 anywhere in h0.
            units = [
                ("vt", 2), ("vt", 3), ("q", 0, 1), ("vt", 4), ("vt", 5),
                ("q", 0, 2), ("k", 1, 0), ("k", 1, 1), ("k", 1, 2), ("k", 1, 3),
                ("q", 0, 3), ("k", 1, 4), ("q", 1, 0), ("q", 1, 1), ("q", 1, 2),
                ("q", 0, 4), ("q", 1, 3), ("q", 1, 4),
            ]

            def emit_unit(u):
                if u[0] == "vt":
                    emit_vt(mixp, u[1], tag="mix")
                else:
                    name, h, p = u
                    emit_proj(mixp, name, h, *qpieces[p][:2], tag="mix")

            # ---- attention + normalize + output projection, pipelined ----
            with (
                tc.tile_pool(name="spsum", bufs=2, space="PSUM") as spsum,
                tc.tile_pool(name="opsum", bufs=1, space="PSUM") as opsum,
                tc.tile_pool(name="mixp", bufs=1, space="PSUM") as mixp,
            ):
                den_dram = dramp.tile([HPC, N], BF16, tag="dend")
                inv_dram = dramp.tile([HPC, N], F32, tag="invd")

                def emit_wo(j, pool):
                    tag = "mix" if pool is mixp else "o"
                    wp = pool.tile([128, 512], F32, tag=tag, name="wp")
                    nc.tensor.matmul(
                        wp[:, :C],
                        on2[:, 128 * j : 128 * (j + 1)],
                        wo2,
                        start=True,
                        stop=True,
                    )
                    ot = outp.tile([128, C], F32, tag="ot")
                    nc.vector.tensor_tensor(
                        out=ot, in0=wp[:, :C], in1=bo_rep, op=mybir.AluOpType.add
                    )
                    nc.sync.dma_start(
                        out=out_d[128 * j : 128 * (j + 1), :], in_=ot
                    )

                def emit_wo_split(j, jj, pool):
                    # unnormalized per-head Wo + per-partition 1/den columns
                    tag = "mix" if pool is mixp else "o"
                    wp = pool.tile([128, 512], F32, tag=tag, name="wp")
                    for h in range(HPC):
                        nc.tensor.matmul(
                            wp[:, C * h : C * h + C],
                            oc[h][:HD, 128 * j : 128 * (j + 1)],
                            wo2h[:, h, :],
                            start=True,
                            stop=True,
                        )
                    t1 = outp.tile([128, C], F32, tag="ot")
                    nc.vector.scalar_tensor_tensor(
                        out=t1,
                        in0=wp[:, :C],
                        scalar=dinv_last[0][:, jj : jj + 1],
                        in1=bo_rep,
                        op0=mybir.AluOpType.mult,
                        op1=mybir.AluOpType.add,
                    )
                    ot = outp.tile([128, C], F32, tag="ot")
                    nc.vector.scalar_tensor_tensor(
                        out=ot,
                        in0=wp[:, C : 2 * C],
                        scalar=dinv_last[1][:, jj : jj + 1],
                        in1=t1,
                        op0=mybir.AluOpType.mult,
                        op1=mybir.AluOpType.add,
                    )
                    nc.sync.dma_start(
                        out=out_d[128 * j : 128 * (j + 1), :], in_=ot
                    )

                def emit_pv(h, chunks, ex, o_ps, qln):
                    # chunk j accumulates into PV band j%2 (PSUM partitions
                    # 64*(j%2)+[0,33)); the two bands' column-group matmuls
                    # run concurrently on the PE.
                    for i, j in enumerate(chunks):
                        bb = j % 2
                        nc.tensor.matmul(
                            o_ps[64 * bb : 64 * bb + 33, :qln],
                            vhat[:, j, h, :],
                            ex[:, qln * i : qln * i + qln],
                            start=(j < 2),
                            stop=(j >= NCH - 2),
                            tile_position=(0, 64 * bb),
                        )

                pend = []  # output-projection chunks ready to emit

                def finish_piece(h, qoff, qln, o_ps):
                    # instruction, so stage band1 in SBUF then add band0.
                    # (row 32 = softmax denominator)
                    ob1 = obp.tile([33, NQB], BF16, tag="ob1")
                    nc.vector.tensor_copy(
                        out=ob1[:, :qln], in_=o_ps[64:97, :qln]
                    )
                    nc.vector.tensor_tensor(
                        out=oc[h][:, qoff : qoff + qln],
                        in0=o_ps[0:33, :qln],
                        in1=ob1[:, :qln],
                        op=mybir.AluOpType.add,
                    )
                    # dense reciprocal via DRAM bounce (gpsimd queue FIFO)
                    nc.gpsimd.dma_start(
                        out=den_dram[h : h + 1, qoff : qoff + qln],
                        in_=oc[h][HD : HD + 1, qoff : qoff + qln],
                    )
                    nd = qln // 128
                    dsrc = den_dram[h : h + 1, qoff : qoff + qln]
                    if qoff == LASTQ:
                        # chunk-column layout feeding the split-Wo tail:
                        # partition p, col jj <- token qoff + 128*jj + p
                        dent_bf = invp.tile(
                            [128, 4], BF16, tag="invb", name="dent_bf"
                        )
                        nc.gpsimd.dma_start(
                            out=dent_bf[:, :nd],
                            in_=bass.AP(
                                tensor=dsrc.tensor,
                                offset=dsrc.offset,
                                ap=[[1, 128], [128, nd]],
                            ),
                        )
                        nc.vector.tensor_copy(
                            out=dinv_last[h][:, :nd], in_=dent_bf[:, :nd]
                        )
                        nc.vector.reciprocal(
                            out=dinv_last[h][:, :nd],
                            in_=dinv_last[h][:, :nd],
                        )
                        return  # no broadcast, no normalize, no pend
                    dent_bf = invp.tile(
                        [128, 4], BF16, tag="invb", name="dent_bf"
                    )
                    nc.gpsimd.dma_start(
                        out=dent_bf[:, :nd],
                        in_=bass.AP(
                            tensor=dsrc.tensor,
                            offset=dsrc.offset,
                            ap=[[nd, 128], [1, nd]],
                        ),
                    )
                    dinv = invp.tile([128, 4], F32, tag="invf", name="dinv")
                    nc.vector.tensor_copy(
                        out=dinv[:, :nd], in_=dent_bf[:, :nd]
                    )
                    nc.vector.reciprocal(out=dinv[:, :nd], in_=dinv[:, :nd])
                    idst = inv_dram[h : h + 1, qoff : qoff + qln]
                    nc.gpsimd.dma_start(
                        out=bass.AP(
                            tensor=idst.tensor,
                            offset=idst.offset,
                            ap=[[nd, 128], [1, nd]],
                        ),
                        in_=dinv[:, :nd],
                    )
                    ibc = ibcp.tile([HD, NQB], F32, tag="ibc")
                    src = inv_dram[h : h + 1, qoff : qoff + qln]
                    bc = bass.AP(
                        tensor=src.tensor,
                        offset=src.offset,
                        ap=[[0, HD]] + [list(d) for d in src.ap[1:]],
                    )
                    nc.gpsimd.dma_start(out=ibc[:, :qln], in_=bc)
                    nc.vector.tensor_tensor(
                        out=on2[HD * h : HD * (h + 1), qoff : qoff + qln],
                        in0=oc[h][:HD, qoff : qoff + qln],
                        in1=ibc[:, :qln],
                        op=mybir.AluOpType.mult,
                    )
                    # piece-boundary slot: one more wo during h1
                    if h == 1 and pend:
                        emit_wo(pend.pop(0), mixp)
                    if h == 1:
                        j0 = qoff // 128
                        pend.extend(range(j0, j0 + qln // 128))

                # The final emit_pv of a piece waits on that piece's last exp;
                # emitted at piece end it head-of-line-blocks the next piece's
                # scores in the PE FIFO (~600ns ACTIVATE gap per boundary).
                # Instead, carry it (plus the whole piece epilogue) into the
                # next piece's first group block.
                carry = None
                for h in range(HPC):
                    for qoff, qln in qpieces:
                        cpg = 6 if qln == 256 else 3
                        groups = [
                            list(range(g, min(g + cpg, NCH)))
                            for g in range(0, NCH, cpg)
                        ]
                        o_ps = opsum.tile([128, NQB], F32, tag="o")
                        ex_prev = None
                        chunks_prev = None
                        for gi, chunks in enumerate(groups):
                            s_tri = spsum.tile([128, 3 * NQB], F32, tag="s")
                            for i, j in enumerate(chunks):
                                # chunks sharing a PSUM bank must share a row
                                # band (serialize); distinct banks may differ.
                                a = (i % 3) if qln == NQB else ((i // 2) % 3)
                                nc.tensor.matmul(
                                    s_tri[:, qln * i : qln * i + qln],
                                    k_rep[h][
                                        32 * a : 32 * a + 32,
                                        128 * j : 128 * j + 128,
                                    ],
                                    q_rep[h][
                                        32 * a : 32 * a + 32, qoff : qoff + qln
                                    ],
                                    start=True,
                                    stop=True,
                                )
                            if gi > 0:
                                emit_pv(h, chunks_prev, ex_prev, o_ps, qln)
                            elif carry is not None:
                                ch, cq, cqln, co, cex, cch = carry
                                emit_pv(ch, cch, cex, co, cqln)
                                finish_piece(ch, cq, cqln, co)
                                carry = None
                            if gi >= 1:
                                if h == 0 and units:
                                    emit_unit(units.pop(0))
                                elif pend and gi >= 3:
                                    # >=3: the chunk's normalize chain (~3us)
                                    # must be done or the Wo matmul would
                                    # head-of-line-block the PE FIFO.
                                    emit_wo(pend.pop(0), mixp)
                            ex = expp.tile([128, 3 * NQB], BF16, tag="ex")
                            ncol = qln * len(chunks)
                            nc.scalar.activation(
                                out=ex[:, :ncol],
                                in_=s_tri[:, :ncol],
                                func=mybir.ActivationFunctionType.Exp,
                                scale=SCALE,
                            )
                            ex_prev = ex
                            chunks_prev = chunks
                        carry = (h, qoff, qln, o_ps, ex_prev, chunks_prev)
                # final piece epilogue + leftover output projections,
                # alternating the two free PSUM pools so the chains pipeline
                ch, cq, cqln, co, cex, cch = carry
                emit_pv(ch, cch, cex, co, cqln)
                finish_piece(ch, cq, cqln, co)
                wi = 0
                while pend:
                    emit_wo(pend.pop(0), (mixp, opsum)[wi % 2])
                    wi += 1
                for jj in range(2):
                    emit_wo_split(LASTQ // 128 + jj, jj, (mixp, opsum)[wi % 2])
                    wi += 1

    if split:
        _split_excess_waits(nc)
    return nc


def kernel(x, Wq, bq, Wk, bk, Wv, bv, Wo, bo):
    global LAST_RESULTS, _CACHED_NC
    x = np.asarray(x, dtype=np.float32)
    Wq = np.asarray(Wq, dtype=np.float32)
    Wk = np.asarray(Wk, dtype=np.float32)
    Wv = np.asarray(Wv, dtype=np.float32)
    Wo = np.asarray(Wo, dtype=np.float32)
    bq = np.asarray(bq, dtype=np.float32)
    bk = np.asarray(bk, dtype=np.float32)
    bv = np.asarray(bv, dtype=np.float32)
    bo = np.asarray(bo, dtype=np.float32)

    def wrep(W, g):
        # [C, 2, 96]: head h cols = W[64g+32h : 64g+32h+32, :].T tiled 3x
        out = np.empty((C, HPC, 96), np.float32)
        for h in range(HPC):
            blk = W[GD * g + HD * h : GD * g + HD * (h + 1), :].T  # [C, 32]
            out[:, h, :] = np.tile(blk, (1, 3))
        return out

    def b3(bvec, g, h):
        return np.tile(bvec[GD * g + HD * h : GD * g + HD * (h + 1)], 3)

    xf = x.reshape(B, C, N)
    in_maps = []
    for core in range(NC):
        b = core // 4
        g = core % 4
        sl = slice(GD * g, GD * (g + 1))
        w_all = np.concatenate(
            [
                wrep(Wq, g).reshape(C, 192),
                wrep(Wk, g).reshape(C, 192),
                Wv[sl, :].T,
            ],
            axis=1,
        )
        b4 = np.stack(
            [b3(bq, g, 0), b3(bq, g, 1), b3(bk, g, 0), b3(bk, g, 1)], axis=1
        )
        # bv 3 replicas (for batched V^T drains) then bo
        brep = np.concatenate(
            [np.tile(bv[sl], 3), bo if g == 0 else np.zeros(C, np.float32)]
        ).reshape(1, 3 * GD + C)
        in_maps.append(
            {
                "x": np.ascontiguousarray(xf[b].astype(NPBF16)),
                "w_all": np.ascontiguousarray(w_all.astype(NPBF16)),
                "b4": np.ascontiguousarray(b4),
                "brep": np.ascontiguousarray(brep),
                "wo_t": np.ascontiguousarray(Wo[:, sl].T.astype(NPBF16)),
            }
        )

    if _CACHED_NC is None:
        _CACHED_NC = build_nc()
    res = run_bass_kernel_spmd(_CACHED_NC, in_maps, core_ids=list(range(NC)))
    LAST_RESULTS = res

    out = np.zeros((B, C, N), dtype=np.float32)
    for core in range(NC):
        out[core // 4] += res.results[core]["out_t"].T
    return out.reshape(B, C, 48, 48)


# revision 25
# speedup vs baseline: 1.0330x; 1.0120x over previous
"""CrossSpectralAttention Trainium2 kernel (bf16, pipelined, v5).

Multi-head attention over 48x48 spatial tokens: B=2, C=256, 8 heads x
head_dim 32, N=2304 tokens. Sharded over 8 NeuronCores as 2 batches x 4
head-groups (2 heads per core). Each core computes its heads' Q/K/V
projections, attention, and a partial output projection (column slice of
Wo); the host sums the 4 partials per batch.

The wall-clock floor is the softmax exp: 2 heads x N^2 = 10.6M elements
through ScalarE at 1 elem/cycle/lane = ~83us busy. Everything else is
arranged to keep ScalarE saturated end-to-end:

- Scores are 3-band row-tiled (q/k replicated 3x on 96 partitions); exp
  runs once per group on the full [128, 1536] PSUM block. The 256-wide
  tail q-piece uses 6-chunk groups -> same 1536-col call density.
  NOTE: EXP ACTIVATE with scale=1.0 is ~20% SLOWER than scale!=1.0 on
  this silicon (measured 1848ns vs 1540ns for identical [128,1536]
  calls), so the softmax 1/sqrt(d) scale stays in the instruction.
- PV is 2-band col-tiled (tile_position=(0,0)/(0,64)): chunk j
  accumulates into PSUM partitions 64*(j%2)+[0,33); the drain is one DVE
  copy (band1 -> SBUF) + one DVE add -> oc bf16.
- Softmax denominators ride as a ones-column in vhat. Reciprocals are
  computed DENSELY ([1,qln] den row -> DRAM -> [128,qln/128] -> 1/x ->
  DRAM -> stride-0 broadcast read); a [1,512] reciprocal would be
  1-lane-bound. All hops ride the gpsimd DMA queue (FIFO-ordered).
- A junk-matmul burst at t~10us warms the PE (HAM K=8) before the real
  projections, and a dummy exp pulls the ACT table load into the DMA
  window.
- V^T projection and the h1/q1/k1 projections stream through the h0
  attention slots; slots at piece boundaries absorb the units whose
  deadlines conflict with in-piece slots.
- Output-projection (Wo) chunks are consumed >= 3 groups into the next
  piece so their normalize chains never head-of-line-block the PE FIFO.
- The final 256-token piece skips the broadcast/normalize chain
  entirely: Wo runs per-head on unnormalized oc and the two per-head
  1/den columns are applied per-PARTITION by scalar_tensor_tensor on
  the Wo output (2 DMA hops instead of 4 on the exposed tail).
"""

import numpy as np
import ml_dtypes

import concourse.bass as bass
import concourse.tile as tile
from concourse import mybir
from concourse.bass_utils import run_bass_kernel_spmd

B = 2
C = 256
N = 2304  # 48*48
NH = 8  # total heads
HPC = 2  # heads per core
HD = 32  # head dim
GD = HPC * HD  # 64 dims per core
NC = 8  # cores
NQB = 512  # query-block size for attention
NCH = N // 128  # 18 m-chunks
SCALE = float(HD) ** -0.5

F32 = mybir.dt.float32
BF16 = mybir.dt.bfloat16
NPBF16 = ml_dtypes.bfloat16

LAST_RESULTS = None  # BassKernelResults of the most recent run (for test.py)
_CACHED_NC = None


def _split_excess_waits(nc, max_waits=1):
    """This walrus build allows a single sync-wait per instruction; move
    excess waits onto same-engine NoOps inserted before the instruction."""
    state = {"uid": 0}

    def fix_block(b):
        i = 0
        insts = b.instructions
        while i < len(insts):
            inst = insts[i]
            for sub in getattr(inst, "blocks", None) or []:
                fix_block(sub)
            si = inst.sync_info
            if si is not None and si.on_wait and len(si.on_wait) > max_waits:
                waits = list(si.on_wait)
                keep, extra = waits[:max_waits], waits[max_waits:]
                inst.sync_info = mybir.SyncInfo(
                    on_wait=keep, on_update=list(si.on_update or [])
                )
                nops = []
                for j in range(0, len(extra), max_waits):
                    nop = mybir.InstNoOp(name=f"WSPLIT-{state['uid']}", ins=[], outs=[])
                    state["uid"] += 1
                    nop.engine = inst.engine
                    nop.sync_info = mybir.SyncInfo(
                        on_wait=extra[j : j + max_waits], on_update=[]
                    )
                    nops.append(nop)
                for k, nop in enumerate(nops):
                    insts.insert(i + k, nop)
                i += len(nops)
            i += 1

    for f in nc.m.functions:
        for b in f.blocks:
            fix_block(b)


def _pieces(total, piece):
    out = []
    o = 0
    while o < total:
        ln = min(piece, total - o)
        out.append((o, ln))
        o += ln
    return out


def build_nc(split=True):
    nc = bass.Bass()

    # wq_t/wk_t carry 3 replicated copies of each head's 32 W^T-columns so
    # the projection matmul writes the 3-band PE layout directly.
    # w_all packs wq|wk|wv column-wise: [0:192] q (h-major), [192:384] k,
    # [384:448] v - one DMA for all three projection weights.
    x_d = nc.dram_tensor("x", [C, N], BF16, kind="ExternalInput")
    wall_d = nc.dram_tensor("w_all", [C, 448], BF16, kind="ExternalInput")
    # b4 packs bq|bk per head column-wise: cols q0,q1,k0,k1 (3-band layout)
    b4_d = nc.dram_tensor("b4", [96, 4], F32, kind="ExternalInput")
    # brep packs bv (x3 replicas for batched V^T drains) | bo row-wise
    brep_d = nc.dram_tensor("brep", [1, 3 * GD + C], F32, kind="ExternalInput")
    wo_d = nc.dram_tensor("wo_t", [GD, C], BF16, kind="ExternalInput")
    out_d = nc.dram_tensor("out_t", [N, C], F32, kind="ExternalOutput")

    qpieces = _pieces(N, NQB)
    LASTQ = qpieces[-1][0]  # 2048

    with tile.TileContext(nc) as tc:
        with (
            tc.tile_pool(name="singles", bufs=1) as singles,
            tc.tile_pool(name="expp", bufs=3) as expp,
            tc.tile_pool(name="invp", bufs=4) as invp,
            tc.tile_pool(name="ibcp", bufs=2) as ibcp,
            tc.tile_pool(name="obp", bufs=2) as obp,
            tc.tile_pool(name="outp", bufs=3) as outp,
            tc.tile_pool(name="dram", bufs=1, space="DRAM") as dramp,
        ):
            # ---- inputs to SBUF; first slab small so projections start early
            x_sb = singles.tile([128, 2, N], BF16)
            xr = x_d.rearrange("(c p) n -> p c n", p=128)
            w_sb = singles.tile([128, 2, 448], BF16)
            nc.gpsimd.dma_start(
                out=w_sb, in_=wall_d.rearrange("(c p) d -> p c d", p=128)
            )
            nc.sync.dma_start(out=x_sb[:, :, :512], in_=xr[:, :, :512])
            nc.sync.dma_start(out=x_sb[:, :, 512:1408], in_=xr[:, :, 512:1408])
            nc.sync.dma_start(out=x_sb[:, :, 1408:], in_=xr[:, :, 1408:])
            b4 = singles.tile([96, 4], F32)
            nc.gpsimd.dma_start(out=b4, in_=b4_d[:, :])
            # Wo^T for both heads (one 64-deep matmul)...
            wo2 = singles.tile([GD, C], BF16)
            nc.scalar.dma_start(out=wo2, in_=wo_d[:, :])
            # ...and per-head at base partition 0 (split-Wo tail path)
            wo2h = singles.tile([HD, HPC, C], BF16)
            nc.scalar.dma_start(
                out=wo2h, in_=wo_d.rearrange("(h d) c -> d h c", h=HPC)
            )
            # bv (3 replicas) | bo replicated across partitions
            brep = singles.tile([128, 3 * GD + C], F32)
            nc.scalar.dma_start(
                out=brep,
                in_=bass.AP(
                    tensor=brep_d, offset=0, ap=[[0, 128], [1, 3 * GD + C]]
                ),
            )
            bv_rep3 = brep[:, : 3 * GD]
            bo_rep = brep[:, 3 * GD :]
            wof = {"q": 0, "k": 192}
            bcol = {("q", 0): 0, ("q", 1): 1, ("k", 0): 2, ("k", 1): 3}

            # dummy exp pulls the ACT table load into the DMA window
            dum = invp.tile([1, 4], F32, tag="dum", name="dum")
            nc.gpsimd.memset(dum, 0.0)
            nc.scalar.activation(
                out=dum, in_=dum, func=mybir.ActivationFunctionType.Exp,
                scale=SCALE,
            )
            # q/k in 3-band replicated layout [96, N] per head
            q_rep = [
                singles.tile([96, N], BF16, name=f"qrep{h}", tag=f"qrep{h}")
                for h in range(HPC)
            ]
            k_rep = [
                singles.tile([96, N], BF16, name=f"krep{h}", tag=f"krep{h}")
                for h in range(HPC)
            ]
            dest = {"q": q_rep, "k": k_rep}

            # vhat[:, j, h, :] = [V_t_h(chunk j) | 1] per head
            vhat = singles.tile([128, NCH, HPC, HD + 1], BF16)
            nc.gpsimd.memset(vhat[:, :, :, HD : HD + 1], 1.0)

            # oc[h] rows 0..31: head h's unnormalized output, row 32: its
            # softmax denominator. on2 rows 32h..32h+31: normalized.
            oc = [
                singles.tile([HD + 1, N], BF16, name=f"oc{h}", tag=f"oc{h}")
                for h in range(HPC)
            ]
            on2 = singles.tile([GD, N], BF16)
            # per-head 1/den for the split-Wo tail (chunk-column layout);
            # cols 0-3: piece-1536 chunks 12-15, cols 4-5: piece-2048 16-17
            dinv_last = [
                singles.tile([128, 8], F32, name=f"dinvl{h}", tag=f"dinvl{h}")
                for h in range(HPC)
            ]

            def emit_proj(psum_pool, name, h, off, ln, tag="proj"):
                ps = psum_pool.tile([96, 512], F32, tag=tag, name="ps")
                for c in range(2):
                    nc.tensor.matmul(
                        ps[:, :ln],
                        w_sb[:, c, wof[name] + 96 * h : wof[name] + 96 * h + 96],
                        x_sb[:, c, off : off + ln],
                        start=(c == 0),
                        stop=(c == 1),
                    )
                nc.vector.tensor_scalar(
                    out=dest[name][h][:, off : off + ln],
                    in0=ps[:, :ln],
                    scalar1=b4[:, bcol[(name, h)] : bcol[(name, h)] + 1],
                    scalar2=None,
                    op0=mybir.AluOpType.add,
                )

            def emit_vt(psum_pool, b, tag="proj"):
                # V^T for chunks 3b..3b+2, one PSUM tile + one DVE drain
                tp = psum_pool.tile([128, 3 * GD], F32, tag=tag, name="tp")
                for jj in range(3):
                    j = 3 * b + jj
                    for c in range(2):
                        nc.tensor.matmul(
                            tp[:, GD * jj : GD * (jj + 1)],
                            x_sb[:, c, 128 * j : 128 * (j + 1)],
                            w_sb[:, c, 384:448],
                            start=(c == 0),
                            stop=(c == 1),
                        )
                nc.vector.tensor_tensor(
                    out=vhat[:, 3 * b : 3 * b + 3, :, :HD],
                    in0=tp.rearrange("p (j h d) -> p j h d", j=3, h=HPC),
                    in1=bv_rep3.rearrange("p (j h d) -> p j h d", j=3, h=HPC),
                    op=mybir.AluOpType.add,
                )

            # ---- upfront: PE warm-up burst, k(h0) full, q(h0)p0, V^T 0-2
            with tc.tile_pool(name="proj_psum", bufs=4, space="PSUM") as proj_psum:
                emit_proj(proj_psum, "k", 0, *qpieces[0][:2])
                emit_proj(proj_psum, "q", 0, *qpieces[0][:2])
                for off, ln in qpieces[1:]:
                    emit_proj(proj_psum, "k", 0, off, ln)
                emit_vt(proj_psum, 0)
                emit_vt(proj_psum, 1)

            # remaining projection work, streamed through the h0 attention
            # slots. Vt_b must be emitted before PV(g_b) (block g_b+1); q0
            # piece p+1 lands in piece p's BOUNDARY slot (its deadline
            # conflicts with the in-piece Vt slots); k1/q1 anywhere in h0.
            units = [
                ("vt", 2), ("vt", 3), ("q", 0, 1), ("vt", 4), ("vt", 5),
                ("q", 0, 2), ("k", 1, 0), ("k", 1, 1), ("k", 1, 2), ("k", 1, 3),
                ("q", 0, 3), ("k", 1, 4), ("q", 1, 0), ("q", 1, 1), ("q", 1, 2),
                ("q", 0, 4), ("q", 1, 3), ("q", 1, 4),
            ]

            def emit_unit(u):
                if u[0] == "vt":
                    emit_vt(mixp, u[1], tag="mix")
                else:
                    name, h, p = u
                    emit_proj(mixp, name, h, *qpieces[p][:2], tag="mix")

            # ---- attention + normalize + output projection, pipelined ----
            with (
                tc.tile_pool(name="spsum", bufs=2, space="PSUM") as spsum,
                tc.tile_pool(name="opsum", bufs=1, space="PSUM") as opsum,
                tc.tile_pool(name="mixp", bufs=1, space="PSUM") as mixp,
            ):
                den_dram = dramp.tile([HPC, N], BF16, tag="dend")
                inv_dram = dramp.tile([HPC, N], F32, tag="invd")

                def emit_wo(j, pool):
                    tag = "mix" if pool is mixp else "o"
                    wp = pool.tile([128, 512], F32, tag=tag, name="wp")
                    nc.tensor.matmul(
                        wp[:, :C],
                        on2[:, 128 * j : 128 * (j + 1)],
                        wo2,
                        start=True,
                        stop=True,
                    )
                    ot = outp.tile([128, C], F32, tag="ot")
                    nc.vector.tensor_tensor(
                        out=ot, in0=wp[:, :C], in1=bo_rep, op=mybir.AluOpType.add
                    )
                    nc.sync.dma_start(
                        out=out_d[128 * j : 128 * (j + 1), :], in_=ot
                    )

                def emit_wo_split(j, jj, pool):
                    # unnormalized per-head Wo + per-partition 1/den columns
                    tag = "mix" if pool is mixp else "o"
                    wp = pool.tile([128, 512], F32, tag=tag, name="wp")
                    for h in range(HPC):
                        nc.tensor.matmul(
                            wp[:, C * h : C * h + C],
                            oc[h][:HD, 128 * j : 128 * (j + 1)],
                            wo2h[:, h, :],
                            start=True,
                            stop=True,
                        )
                    t1 = outp.tile([128, C], F32, tag="ot")
                    nc.vector.scalar_tensor_tensor(
                        out=t1,
                        in0=wp[:, :C],
                        scalar=dinv_last[0][:, jj : jj + 1],
                        in1=bo_rep,
                        op0=mybir.AluOpType.mult,
                        op1=mybir.AluOpType.add,
                    )
                    ot = outp.tile([128, C], F32, tag="ot")
                    nc.vector.scalar_tensor_tensor(
                        out=ot,
                        in0=wp[:, C : 2 * C],
                        scalar=dinv_last[1][:, jj : jj + 1],
                        in1=t1,
                        op0=mybir.AluOpType.mult,
                        op1=mybir.AluOpType.add,
                    )
                    nc.sync.dma_start(
                        out=out_d[128 * j : 128 * (j + 1), :], in_=ot
                    )

                def emit_pv(h, chunks, ex, o_ps, qln):
                    # chunk j accumulates into PV band j%2 (PSUM partitions
                    # 64*(j%2)+[0,33)); the two bands' column-group matmuls
                    # run concurrently on the PE.
                    for i, j in enumerate(chunks):
                        bb = j % 2
                        nc.tensor.matmul(
                            o_ps[64 * bb : 64 * bb + 33, :qln],
                            vhat[:, j, h, :],
                            ex[:, qln * i : qln * i + qln],
                            start=(j < 2),
                            stop=(j >= NCH - 2),
                            tile_position=(0, 64 * bb),
                        )

                pend = []  # output-projection chunks ready to emit

                def finish_piece(h, qoff, qln, o_ps):
                    # instruction, so stage band1 in SBUF then add band0.
                    # (row 32 = softmax denominator)
                    ob1 = obp.tile([33, NQB], BF16, tag="ob1")
                    nc.vector.tensor_copy(
                        out=ob1[:, :qln], in_=o_ps[64:97, :qln]
                    )
                    nc.vector.tensor_tensor(
                        out=oc[h][:, qoff : qoff + qln],
                        in0=o_ps[0:33, :qln],
                        in1=ob1[:, :qln],
                        op=mybir.AluOpType.add,
                    )
                    # dense reciprocal via DRAM bounce (gpsimd queue FIFO)
                    nc.gpsimd.dma_start(
                        out=den_dram[h : h + 1, qoff : qoff + qln],
                        in_=oc[h][HD : HD + 1, qoff : qoff + qln],
                    )
                    nd = qln // 128
                    dsrc = den_dram[h : h + 1, qoff : qoff + qln]
                    # piece-boundary slot: one more wo during h1
                    if h == 1 and pend:
                        emit_wo(pend.pop(0), mixp)
                    if qoff >= 1536:
                        # chunk-column layout feeding the split-Wo tail:
                        # partition p, col jj <- token qoff + 128*jj + p
                        coff = 0 if qoff == 1536 else 4
                        dent_bf = invp.tile(
                            [128, 4], BF16, tag="invb", name="dent_bf"
                        )
                        nc.gpsimd.dma_start(
                            out=dent_bf[:, :nd],
                            in_=bass.AP(
                                tensor=dsrc.tensor,
                                offset=dsrc.offset,
                                ap=[[1, 128], [128, nd]],
                            ),
                        )
                        nc.vector.tensor_copy(
                            out=dinv_last[h][:, coff : coff + nd],
                            in_=dent_bf[:, :nd],
                        )
                        nc.vector.reciprocal(
                            out=dinv_last[h][:, coff : coff + nd],
                            in_=dinv_last[h][:, coff : coff + nd],
                        )
                        return  # no broadcast, no normalize, no pend
                    dent_bf = invp.tile(
                        [128, 4], BF16, tag="invb", name="dent_bf"
                    )
                    nc.gpsimd.dma_start(
                        out=dent_bf[:, :nd],
                        in_=bass.AP(
                            tensor=dsrc.tensor,
                            offset=dsrc.offset,
                            ap=[[nd, 128], [1, nd]],
                        ),
                    )
                    dinv = invp.tile([128, 4], F32, tag="invf", name="dinv")
                    nc.vector.tensor_copy(
                        out=dinv[:, :nd], in_=dent_bf[:, :nd]
                    )
                    nc.vector.reciprocal(out=dinv[:, :nd], in_=dinv[:, :nd])
                    idst = inv_dram[h : h + 1, qoff : qoff + qln]
                    nc.gpsimd.dma_start(
                        out=bass.AP(
                            tensor=idst.tensor,
                            offset=idst.offset,
                            ap=[[nd, 128], [1, nd]],
                        ),
                        in_=dinv[:, :nd],
                    )
                    ibc = ibcp.tile([HD, NQB], F32, tag="ibc")
                    src = inv_dram[h : h + 1, qoff : qoff + qln]
                    bc = bass.AP(
                        tensor=src.tensor,
                        offset=src.offset,
                        ap=[[0, HD]] + [list(d) for d in src.ap[1:]],
                    )
                    nc.gpsimd.dma_start(out=ibc[:, :qln], in_=bc)
                    nc.vector.tensor_tensor(
                        out=on2[HD * h : HD * (h + 1), qoff : qoff + qln],
                        in0=oc[h][:HD, qoff : qoff + qln],
                        in1=ibc[:, :qln],
                        op=mybir.AluOpType.mult,
                    )
                    if h == 1:
                        j0 = qoff // 128
                        pend.extend(range(j0, j0 + qln // 128))

                # The final emit_pv of a piece waits on that piece's last exp;
                # emitted at piece end it head-of-line-blocks the next piece's
                # scores in the PE FIFO (~600ns ACTIVATE gap per boundary).
                # Instead, carry it (plus the whole piece epilogue) into the
                # next piece's first group block.
                carry = None
                for h in range(HPC):
                    for qoff, qln in qpieces:
                        cpg = 6 if qln == 256 else 3
                        groups = [
                            list(range(g, min(g + cpg, NCH)))
                            for g in range(0, NCH, cpg)
                        ]
                        o_ps = opsum.tile([128, NQB], F32, tag="o")
                        ex_prev = None
                        chunks_prev = None
                        for gi, chunks in enumerate(groups):
                            s_tri = spsum.tile([128, 3 * NQB], F32, tag="s")
                            for i, j in enumerate(chunks):
                                # chunks sharing a PSUM bank must share a row
                                # band (serialize); distinct banks may differ.
                                a = (i % 3) if qln == NQB else ((i // 2) % 3)
                                nc.tensor.matmul(
                                    s_tri[:, qln * i : qln * i + qln],
                                    k_rep[h][
                                        32 * a : 32 * a + 32,
                                        128 * j : 128 * j + 128,
                                    ],
                                    q_rep[h][
                                        32 * a : 32 * a + 32, qoff : qoff + qln
                                    ],
                                    start=True,
                                    stop=True,
                                )
                            if gi > 0:
                                emit_pv(h, chunks_prev, ex_prev, o_ps, qln)
                            elif carry is not None:
                                ch, cq, cqln, co, cex, cch = carry
                                emit_pv(ch, cch, cex, co, cqln)
                                finish_piece(ch, cq, cqln, co)
                                carry = None
                            if gi >= 1:
                                if h == 0 and units:
                                    emit_unit(units.pop(0))
                                elif pend and gi >= 3:
                                    # >=3: the chunk's normalize chain (~3us)
                                    # must be done or the Wo matmul would
                                    # head-of-line-block the PE FIFO.
                                    emit_wo(pend.pop(0), mixp)
                            ex = expp.tile([128, 3 * NQB], BF16, tag="ex")
                            ncol = qln * len(chunks)
                            nc.scalar.activation(
                                out=ex[:, :ncol],
                                in_=s_tri[:, :ncol],
                                func=mybir.ActivationFunctionType.Exp,
                                scale=SCALE,
                            )
                            ex_prev = ex
                            chunks_prev = chunks
                        carry = (h, qoff, qln, o_ps, ex_prev, chunks_prev)
                # final piece epilogue + leftover output projections,
                # alternating the two free PSUM pools so the chains pipeline
                ch, cq, cqln, co, cex, cch = carry
                emit_pv(ch, cch, cex, co, cqln)
                finish_piece(ch, cq, cqln, co)
                wi = 0
                while pend:
                    emit_wo(pend.pop(0), (mixp, opsum)[wi % 2])
                    wi += 1
                for jj in range(6):
                    emit_wo_split(12 + jj, jj, (mixp, opsum)[wi % 2])
                    wi += 1

    if split:
        _split_excess_waits(nc)
    return nc


def kernel(x, Wq, bq, Wk, bk, Wv, bv, Wo, bo):
    global LAST_RESULTS, _CACHED_NC
    x = np.asarray(x, dtype=np.float32)
    Wq = np.asarray(Wq, dtype=np.float32)
    Wk = np.asarray(Wk, dtype=np.float32)
    Wv = np.asarray(Wv, dtype=np.float32)
    Wo = np.asarray(Wo, dtype=np.float32)
    bq = np.asarray(bq, dtype=np.float32)
    bk = np.asarray(bk, dtype=np.float32)
    bv = np.asarray(bv, dtype=np.float32)
    bo = np.asarray(bo, dtype=np.float32)

    def wrep(W, g):
        # [C, 2, 96]: head h cols = W[64g+32h : 64g+32h+32, :].T tiled 3x
        out = np.empty((C, HPC, 96), np.float32)
        for h in range(HPC):
            blk = W[GD * g + HD * h : GD * g + HD * (h + 1), :].T  # [C, 32]
            out[:, h, :] = np.tile(blk, (1, 3))
        return out

    def b3(bvec, g, h):
        return np.tile(bvec[GD * g + HD * h : GD * g + HD * (h + 1)], 3)

    xf = x.reshape(B, C, N)
    in_maps = []
    for core in range(NC):
        b = core // 4
        g = core % 4
        sl = slice(GD * g, GD * (g + 1))
        w_all = np.concatenate(
            [
                wrep(Wq, g).reshape(C, 192),
                wrep(Wk, g).reshape(C, 192),
                Wv[sl, :].T,
            ],
            axis=1,
        )
        b4 = np.stack(
            [b3(bq, g, 0), b3(bq, g, 1), b3(bk, g, 0), b3(bk, g, 1)], axis=1
        )
        # bv 3 replicas (for batched V^T drains) then bo
        brep = np.concatenate(
            [np.tile(bv[sl], 3), bo if g == 0 else np.zeros(C, np.float32)]
        ).reshape(1, 3 * GD + C)
        in_maps.append(
            {
                "x": np.ascontiguousarray(xf[b].astype(NPBF16)),
                "w_all": np.ascontiguousarray(w_all.astype(NPBF16)),
                "b4": np.ascontiguousarray(b4),
                "brep": np.ascontiguousarray(brep),
                "wo_t": np.ascontiguousarray(Wo[:, sl].T.astype(NPBF16)),
            }
        )

    if _CACHED_NC is None:
        _CACHED_NC = build_nc()
    res = run_bass_kernel_spmd(_CACHED_NC, in_maps, core_ids=list(range(NC)))
    LAST_RESULTS = res

    out = np.zeros((B, C, N), dtype=np.float32)
    for core in range(NC):
        out[core // 4] += res.results[core]["out_t"].T
    return out.reshape(B, C, 48, 48)


# revision 26
# speedup vs baseline: 1.0516x; 1.0180x over previous
"""CrossSpectralAttention Trainium2 kernel (bf16, pipelined, v11).

Multi-head attention over 48x48 spatial tokens: B=2, C=256, 8 heads x
head_dim 32, N=2304 tokens. Sharded over 8 NeuronCores as 2 batches x 4
head-groups (2 heads per core). Each core computes its heads' Q/K/V
projections, attention, and a partial output projection (column slice of
Wo); the host sums the 4 partials per batch.

The wall-clock floor is the softmax exp: 2 heads x N^2 = 10.6M elements
through ScalarE at 1 elem/cycle/lane = ~83us busy. Everything else is
arranged to keep ScalarE saturated end-to-end:

- Scores are 3-band row-tiled (q/k replicated 3x on 96 partitions); exp
  runs once per group on the full [128, 1536] PSUM block. The 256-wide
  tail q-piece uses 6-chunk groups -> same 1536-col call density.
  NOTE: EXP ACTIVATE with scale=1.0 is ~20% SLOWER than scale!=1.0 on
  this silicon (measured 1848ns vs 1540ns for identical [128,1536]
  calls), so the softmax 1/sqrt(d) scale stays in the instruction.
- PV is 2-band col-tiled (tile_position=(0,0)/(0,64)): chunk j
  accumulates into PSUM partitions 64*(j%2)+[0,33); the drain is one DVE
  copy (band1 -> SBUF) + one DVE add -> oc bf16.
- Softmax denominators ride as a ones-column in vhat. Reciprocals are
  computed DENSELY ([1,qln] den row -> DRAM -> [128,qln/128] -> 1/x ->
  DRAM -> stride-0 broadcast read); a [1,512] reciprocal would be
  1-lane-bound. All hops ride the gpsimd DMA queue (FIFO-ordered).
- A dummy exp pulls the ~1.5us ACT table load into the input-DMA window.
- V^T projection and the h1/q1/k1 projections stream through the h0
  attention slots; slots at piece boundaries absorb the units whose
  deadlines conflict with in-piece slots.
- Output-projection (Wo) chunks are consumed >= 3 groups into the next
  piece so their normalize chains never head-of-line-block the PE FIFO.
- The final TWO query pieces (tokens 1536..2304, output chunks 12-17)
  skip the broadcast/normalize chain entirely: Wo runs per-head on
  unnormalized oc and the per-head 1/den columns are applied
  per-PARTITION by scalar_tensor_tensor on the Wo output. Their 1/den
  chains need only 2 DMA hops, and the six final output chunks pipeline
  through two alternating PSUM pools right behind the last exp.
- The PV pipeline epilogue of each piece (final PV group + band-reduce +
  reciprocal chain) is carried into the NEXT piece's first group block
  so it never head-of-line-blocks scores in the PE FIFO.
"""

import numpy as np
import ml_dtypes

import concourse.bass as bass
import concourse.tile as tile
from concourse import mybir
from concourse.bass_utils import run_bass_kernel_spmd

B = 2
C = 256
N = 2304  # 48*48
NH = 8  # total heads
HPC = 2  # heads per core
HD = 32  # head dim
GD = HPC * HD  # 64 dims per core
NC = 8  # cores
NQB = 512  # query-block size for attention
NCH = N // 128  # 18 m-chunks
SCALE = float(HD) ** -0.5

F32 = mybir.dt.float32
BF16 = mybir.dt.bfloat16
NPBF16 = ml_dtypes.bfloat16

LAST_RESULTS = None  # BassKernelResults of the most recent run (for test.py)
_CACHED_NC = None


def _split_excess_waits(nc, max_waits=1):
    """This walrus build allows a single sync-wait per instruction; move
    excess waits onto same-engine NoOps inserted before the instruction."""
    state = {"uid": 0}

    def fix_block(b):
        i = 0
        insts = b.instructions
        while i < len(insts):
            inst = insts[i]
            for sub in getattr(inst, "blocks", None) or []:
                fix_block(sub)
            si = inst.sync_info
            if si is not None and si.on_wait and len(si.on_wait) > max_waits:
                waits = list(si.on_wait)
                keep, extra = waits[:max_waits], waits[max_waits:]
                inst.sync_info = mybir.SyncInfo(
                    on_wait=keep, on_update=list(si.on_update or [])
                )
                nops = []
                for j in range(0, len(extra), max_waits):
                    nop = mybir.InstNoOp(name=f"WSPLIT-{state['uid']}", ins=[], outs=[])
                    state["uid"] += 1
                    nop.engine = inst.engine
                    nop.sync_info = mybir.SyncInfo(
                        on_wait=extra[j : j + max_waits], on_update=[]
                    )
                    nops.append(nop)
                for k, nop in enumerate(nops):
                    insts.insert(i + k, nop)
                i += len(nops)
            i += 1

    for f in nc.m.functions:
        for b in f.blocks:
            fix_block(b)


def _pieces(total, piece):
    out = []
    o = 0
    while o < total:
        ln = min(piece, total - o)
        out.append((o, ln))
        o += ln
    return out


def build_nc(split=True):
    nc = bass.Bass()

    # wq_t/wk_t carry 3 replicated copies of each head's 32 W^T-columns so
    # the projection matmul writes the 3-band PE layout directly.
    # w_all packs wq|wk|wv column-wise: [0:192] q (h-major), [192:384] k,
    # [384:448] v - one DMA for all three projection weights.
    x_d = nc.dram_tensor("x", [C, N], BF16, kind="ExternalInput")
    wall_d = nc.dram_tensor("w_all", [C, 448], BF16, kind="ExternalInput")
    # b4 packs bq|bk per head column-wise: cols q0,q1,k0,k1 (3-band layout)
    b4_d = nc.dram_tensor("b4", [96, 4], F32, kind="ExternalInput")
    # brep packs bv (x3 replicas for batched V^T drains) | bo row-wise
    brep_d = nc.dram_tensor("brep", [1, 3 * GD + C], F32, kind="ExternalInput")
    wo_d = nc.dram_tensor("wo_t", [GD, C], BF16, kind="ExternalInput")
    out_d = nc.dram_tensor("out_t", [N, C], F32, kind="ExternalOutput")

    qpieces = _pieces(N, NQB)
    LASTQ = qpieces[-1][0]  # 2048

    with tile.TileContext(nc) as tc:
        with (
            tc.tile_pool(name="singles", bufs=1) as singles,
            tc.tile_pool(name="expp", bufs=3) as expp,
            tc.tile_pool(name="invp", bufs=4) as invp,
            tc.tile_pool(name="ibcp", bufs=2) as ibcp,
            tc.tile_pool(name="obp", bufs=2) as obp,
            tc.tile_pool(name="outp", bufs=3) as outp,
            tc.tile_pool(name="dram", bufs=1, space="DRAM") as dramp,
        ):
            # ---- inputs to SBUF; first slab small so projections start early
            x_sb = singles.tile([128, 2, N], BF16)
            xr = x_d.rearrange("(c p) n -> p c n", p=128)
            w_sb = singles.tile([128, 2, 448], BF16)
            nc.gpsimd.dma_start(
                out=w_sb, in_=wall_d.rearrange("(c p) d -> p c d", p=128)
            )
            nc.sync.dma_start(out=x_sb[:, :, :512], in_=xr[:, :, :512])
            nc.sync.dma_start(out=x_sb[:, :, 512:1408], in_=xr[:, :, 512:1408])
            nc.sync.dma_start(out=x_sb[:, :, 1408:], in_=xr[:, :, 1408:])
            b4 = singles.tile([96, 4], F32)
            nc.gpsimd.dma_start(out=b4, in_=b4_d[:, :])
            # Wo^T for both heads (one 64-deep matmul)...
            wo2 = singles.tile([GD, C], BF16)
            nc.scalar.dma_start(out=wo2, in_=wo_d[:, :])
            # ...and per-head at base partition 0 (split-Wo tail path)
            wo2h = singles.tile([HD, HPC, C], BF16)
            nc.scalar.dma_start(
                out=wo2h, in_=wo_d.rearrange("(h d) c -> d h c", h=HPC)
            )
            # bv (3 replicas) | bo replicated across partitions
            brep = singles.tile([128, 3 * GD + C], F32)
            nc.scalar.dma_start(
                out=brep,
                in_=bass.AP(
                    tensor=brep_d, offset=0, ap=[[0, 128], [1, 3 * GD + C]]
                ),
            )
            bv_rep3 = brep[:, : 3 * GD]
            bo_rep = brep[:, 3 * GD :]
            wof = {"q": 0, "k": 192}
            bcol = {("q", 0): 0, ("q", 1): 1, ("k", 0): 2, ("k", 1): 3}

            # dummy exp pulls the ACT table load into the DMA window
            dum = invp.tile([1, 4], F32, tag="dum", name="dum")
            nc.gpsimd.memset(dum, 0.0)
            nc.scalar.activation(
                out=dum, in_=dum, func=mybir.ActivationFunctionType.Exp,
                scale=SCALE,
            )
            # q/k in 3-band replicated layout [96, N] per head
            q_rep = [
                singles.tile([96, N], BF16, name=f"qrep{h}", tag=f"qrep{h}")
                for h in range(HPC)
            ]
            k_rep = [
                singles.tile([96, N], BF16, name=f"krep{h}", tag=f"krep{h}")
                for h in range(HPC)
            ]
            dest = {"q": q_rep, "k": k_rep}

            # vhat[:, j, h, :] = [V_t_h(chunk j) | 1] per head
            vhat = singles.tile([128, NCH, HPC, HD + 1], BF16)
            nc.gpsimd.memset(vhat[:, :, :, HD : HD + 1], 1.0)

            # oc[h] rows 0..31: head h's unnormalized output, row 32: its
            # softmax denominator. on2 rows 32h..32h+31: normalized.
            oc = [
                singles.tile([HD + 1, N], BF16, name=f"oc{h}", tag=f"oc{h}")
                for h in range(HPC)
            ]
            on2 = singles.tile([GD, N], BF16)
            # per-head 1/den for the split-Wo tail (chunk-column layout);
            # cols 0-3: piece-1536 chunks 12-15, cols 4-5: piece-2048 16-17
            dinv_last = [
                singles.tile([128, 8], F32, name=f"dinvl{h}", tag=f"dinvl{h}")
                for h in range(HPC)
            ]

            def emit_proj(psum_pool, name, h, off, ln, tag="proj"):
                ps = psum_pool.tile([96, 512], F32, tag=tag, name="ps")
                for c in range(2):
                    nc.tensor.matmul(
                        ps[:, :ln],
                        w_sb[:, c, wof[name] + 96 * h : wof[name] + 96 * h + 96],
                        x_sb[:, c, off : off + ln],
                        start=(c == 0),
                        stop=(c == 1),
                    )
                nc.vector.tensor_scalar(
                    out=dest[name][h][:, off : off + ln],
                    in0=ps[:, :ln],
                    scalar1=b4[:, bcol[(name, h)] : bcol[(name, h)] + 1],
                    scalar2=None,
                    op0=mybir.AluOpType.add,
                )

            def emit_vt(psum_pool, b, tag="proj"):
                # V^T for chunks 3b..3b+2, one PSUM tile + one DVE drain
                tp = psum_pool.tile([128, 3 * GD], F32, tag=tag, name="tp")
                for jj in range(3):
                    j = 3 * b + jj
                    for c in range(2):
                        nc.tensor.matmul(
                            tp[:, GD * jj : GD * (jj + 1)],
                            x_sb[:, c, 128 * j : 128 * (j + 1)],
                            w_sb[:, c, 384:448],
                            start=(c == 0),
                            stop=(c == 1),
                        )
                nc.vector.tensor_tensor(
                    out=vhat[:, 3 * b : 3 * b + 3, :, :HD],
                    in0=tp.rearrange("p (j h d) -> p j h d", j=3, h=HPC),
                    in1=bv_rep3.rearrange("p (j h d) -> p j h d", j=3, h=HPC),
                    op=mybir.AluOpType.add,
                )

            # ---- upfront: PE warm-up burst, k(h0) full, q(h0)p0, V^T 0-2
            with tc.tile_pool(name="proj_psum", bufs=4, space="PSUM") as proj_psum:
                emit_proj(proj_psum, "k", 0, *qpieces[0][:2])
                emit_proj(proj_psum, "q", 0, *qpieces[0][:2])
                for off, ln in qpieces[1:]:
                    emit_proj(proj_psum, "k", 0, off, ln)
                emit_vt(proj_psum, 0)
                emit_vt(proj_psum, 1)

            # remaining projection work, streamed through the h0 attention
            # slots. Vt_b must be emitted before PV(g_b) (block g_b+1); q0
            # piece p+1 lands in piece p's BOUNDARY slot (its deadline
            # conflicts with the in-piece Vt slots); k1/q1 anywhere in h0.
            units = [
                ("vt", 2), ("vt", 3), ("q", 0, 1), ("vt", 4), ("vt", 5),
                ("q", 0, 2), ("k", 1, 0), ("k", 1, 1), ("k", 1, 2), ("k", 1, 3),
                ("q", 0, 3), ("k", 1, 4), ("q", 1, 0), ("q", 1, 1), ("q", 1, 2),
                ("q", 0, 4), ("q", 1, 3), ("q", 1, 4),
            ]

            def emit_unit(u):
                if u[0] == "vt":
                    emit_vt(mixp, u[1], tag="mix")
                else:
                    name, h, p = u
                    emit_proj(mixp, name, h, *qpieces[p][:2], tag="mix")

            # ---- attention + normalize + output projection, pipelined ----
            with (
                tc.tile_pool(name="spsum", bufs=2, space="PSUM") as spsum,
                tc.tile_pool(name="opsum", bufs=1, space="PSUM") as opsum,
                tc.tile_pool(name="mixp", bufs=1, space="PSUM") as mixp,
            ):
                den_dram = dramp.tile([HPC, N], BF16, tag="dend")
                inv_dram = dramp.tile([HPC, N], F32, tag="invd")

                def emit_wo(j, pool):
                    tag = "mix" if pool is mixp else "o"
                    wp = pool.tile([128, 512], F32, tag=tag, name="wp")
                    nc.tensor.matmul(
                        wp[:, :C],
                        on2[:, 128 * j : 128 * (j + 1)],
                        wo2,
                        start=True,
                        stop=True,
                    )
                    ot = outp.tile([128, C], F32, tag="ot")
                    nc.vector.tensor_tensor(
                        out=ot, in0=wp[:, :C], in1=bo_rep, op=mybir.AluOpType.add
                    )
                    nc.sync.dma_start(
                        out=out_d[128 * j : 128 * (j + 1), :], in_=ot
                    )

                def emit_wo_split(j, jj, pool):
                    # unnormalized per-head Wo + per-partition 1/den columns
                    tag = "mix" if pool is mixp else "o"
                    wp = pool.tile([128, 512], F32, tag=tag, name="wp")
                    for h in range(HPC):
                        nc.tensor.matmul(
                            wp[:, C * h : C * h + C],
                            oc[h][:HD, 128 * j : 128 * (j + 1)],
                            wo2h[:, h, :],
                            start=True,
                            stop=True,
                        )
                    t1 = outp.tile([128, C], F32, tag="ot")
                    nc.vector.scalar_tensor_tensor(
                        out=t1,
                        in0=wp[:, :C],
                        scalar=dinv_last[0][:, jj : jj + 1],
                        in1=bo_rep,
                        op0=mybir.AluOpType.mult,
                        op1=mybir.AluOpType.add,
                    )
                    ot = outp.tile([128, C], F32, tag="ot")
                    nc.vector.scalar_tensor_tensor(
                        out=ot,
                        in0=wp[:, C : 2 * C],
                        scalar=dinv_last[1][:, jj : jj + 1],
                        in1=t1,
                        op0=mybir.AluOpType.mult,
                        op1=mybir.AluOpType.add,
                    )
                    nc.sync.dma_start(
                        out=out_d[128 * j : 128 * (j + 1), :], in_=ot
                    )

                def emit_pv(h, chunks, ex, o_ps, qln):
                    # chunk j accumulates into PV band j%2 (PSUM partitions
                    # 64*(j%2)+[0,33)); the two bands' column-group matmuls
                    # run concurrently on the PE.
                    for i, j in enumerate(chunks):
                        bb = j % 2
                        nc.tensor.matmul(
                            o_ps[64 * bb : 64 * bb + 33, :qln],
                            vhat[:, j, h, :],
                            ex[:, qln * i : qln * i + qln],
                            start=(j < 2),
                            stop=(j >= NCH - 2),
                            tile_position=(0, 64 * bb),
                        )

                pend = []  # output-projection chunks ready to emit

                def finish_piece(h, qoff, qln, o_ps):
                    # instruction, so stage band1 in SBUF then add band0.
                    # (row 32 = softmax denominator)
                    ob1 = obp.tile([33, NQB], BF16, tag="ob1")
                    nc.vector.tensor_copy(
                        out=ob1[:, :qln], in_=o_ps[64:97, :qln]
                    )
                    nc.vector.tensor_tensor(
                        out=oc[h][:, qoff : qoff + qln],
                        in0=o_ps[0:33, :qln],
                        in1=ob1[:, :qln],
                        op=mybir.AluOpType.add,
                    )
                    # dense reciprocal via DRAM bounce (gpsimd queue FIFO)
                    nc.gpsimd.dma_start(
                        out=den_dram[h : h + 1, qoff : qoff + qln],
                        in_=oc[h][HD : HD + 1, qoff : qoff + qln],
                    )
                    nd = qln // 128
                    dsrc = den_dram[h : h + 1, qoff : qoff + qln]
                    # piece-boundary slot: one more wo during h1
                    if h == 1 and pend:
                        emit_wo(pend.pop(0), mixp)
                    if qoff >= 1536:
                        # chunk-column layout feeding the split-Wo tail:
                        # partition p, col jj <- token qoff + 128*jj + p
                        coff = 0 if qoff == 1536 else 4
                        dent_bf = invp.tile(
                            [128, 4], BF16, tag="invb", name="dent_bf"
                        )
                        nc.gpsimd.dma_start(
                            out=dent_bf[:, :nd],
                            in_=bass.AP(
                                tensor=dsrc.tensor,
                                offset=dsrc.offset,
                                ap=[[1, 128], [128, nd]],
                            ),
                        )
                        nc.vector.tensor_copy(
                            out=dinv_last[h][:, coff : coff + nd],
                            in_=dent_bf[:, :nd],
                        )
                        nc.vector.reciprocal(
                            out=dinv_last[h][:, coff : coff + nd],
                            in_=dinv_last[h][:, coff : coff + nd],
                        )
                        return  # no broadcast, no normalize, no pend
                    dent_bf = invp.tile(
                        [128, 4], BF16, tag="invb", name="dent_bf"
                    )
                    nc.gpsimd.dma_start(
                        out=dent_bf[:, :nd],
                        in_=bass.AP(
                            tensor=dsrc.tensor,
                            offset=dsrc.offset,
                            ap=[[nd, 128], [1, nd]],
                        ),
                    )
                    dinv = invp.tile([128, 4], F32, tag="invf", name="dinv")
                    nc.vector.tensor_copy(
                        out=dinv[:, :nd], in_=dent_bf[:, :nd]
                    )
                    nc.vector.reciprocal(out=dinv[:, :nd], in_=dinv[:, :nd])
                    idst = inv_dram[h : h + 1, qoff : qoff + qln]
                    nc.gpsimd.dma_start(
                        out=bass.AP(
                            tensor=idst.tensor,
                            offset=idst.offset,
                            ap=[[nd, 128], [1, nd]],
                        ),
                        in_=dinv[:, :nd],
                    )
                    ibc = ibcp.tile([HD, NQB], F32, tag="ibc")
                    src = inv_dram[h : h + 1, qoff : qoff + qln]
                    bc = bass.AP(
                        tensor=src.tensor,
                        offset=src.offset,
                        ap=[[0, HD]] + [list(d) for d in src.ap[1:]],
                    )
                    nc.gpsimd.dma_start(out=ibc[:, :qln], in_=bc)
                    nc.vector.tensor_tensor(
                        out=on2[HD * h : HD * (h + 1), qoff : qoff + qln],
                        in0=oc[h][:HD, qoff : qoff + qln],
                        in1=ibc[:, :qln],
                        op=mybir.AluOpType.mult,
                    )
                    if h == 1:
                        j0 = qoff // 128
                        pend.extend(range(j0, j0 + qln // 128))

                # The final emit_pv of a piece waits on that piece's last exp;
                # emitted at piece end it head-of-line-blocks the next piece's
                # scores in the PE FIFO (~600ns ACTIVATE gap per boundary).
                # Instead, carry it (plus the whole piece epilogue) into the
                # next piece's first group block.
                carry = None
                for h in range(HPC):
                    for qoff, qln in qpieces:
                        cpg = 6 if qln == 256 else 3
                        groups = [
                            list(range(g, min(g + cpg, NCH)))
                            for g in range(0, NCH, cpg)
                        ]
                        o_ps = opsum.tile([128, NQB], F32, tag="o")
                        ex_prev = None
                        chunks_prev = None
                        for gi, chunks in enumerate(groups):
                            s_tri = spsum.tile([128, 3 * NQB], F32, tag="s")
                            for i, j in enumerate(chunks):
                                # chunks sharing a PSUM bank must share a row
                                # band (serialize); distinct banks may differ.
                                a = (i % 3) if qln == NQB else ((i // 2) % 3)
                                nc.tensor.matmul(
                                    s_tri[:, qln * i : qln * i + qln],
                                    k_rep[h][
                                        32 * a : 32 * a + 32,
                                        128 * j : 128 * j + 128,
                                    ],
                                    q_rep[h][
                                        32 * a : 32 * a + 32, qoff : qoff + qln
                                    ],
                                    start=True,
                                    stop=True,
                                )
                            if gi > 0:
                                emit_pv(h, chunks_prev, ex_prev, o_ps, qln)
                            elif carry is not None:
                                ch, cq, cqln, co, cex, cch = carry
                                emit_pv(ch, cch, cex, co, cqln)
                                finish_piece(ch, cq, cqln, co)
                                carry = None
                            if gi >= 1:
                                if h == 0 and units:
                                    emit_unit(units.pop(0))
                                elif pend and gi >= 3:
                                    # >=3: the chunk's normalize chain (~3us)
                                    # must be done or the Wo matmul would
                                    # head-of-line-block the PE FIFO.
                                    emit_wo(pend.pop(0), mixp)
                            ex = expp.tile([128, 3 * NQB], BF16, tag="ex")
                            ncol = qln * len(chunks)
                            nc.scalar.activation(
                                out=ex[:, :ncol],
                                in_=s_tri[:, :ncol],
                                func=mybir.ActivationFunctionType.Exp,
                                scale=SCALE,
                            )
                            ex_prev = ex
                            chunks_prev = chunks
                        carry = (h, qoff, qln, o_ps, ex_prev, chunks_prev)
                # final piece epilogue + leftover output projections,
                # alternating the two free PSUM pools so the chains pipeline
                ch, cq, cqln, co, cex, cch = carry
                emit_pv(ch, cch, cex, co, cqln)
                finish_piece(ch, cq, cqln, co)
                wi = 0
                while pend:
                    emit_wo(pend.pop(0), (mixp, opsum)[wi % 2])
                    wi += 1
                for jj in range(6):
                    emit_wo_split(12 + jj, jj, (mixp, opsum)[wi % 2])
                    wi += 1

    if split:
        _split_excess_waits(nc)
    return nc


def kernel(x, Wq, bq, Wk, bk, Wv, bv, Wo, bo):
    global LAST_RESULTS, _CACHED_NC
    x = np.asarray(x, dtype=np.float32)
    Wq = np.asarray(Wq, dtype=np.float32)
    Wk = np.asarray(Wk, dtype=np.float32)
    Wv = np.asarray(Wv, dtype=np.float32)
    Wo = np.asarray(Wo, dtype=np.float32)
    bq = np.asarray(bq, dtype=np.float32)
    bk = np.asarray(bk, dtype=np.float32)
    bv = np.asarray(bv, dtype=np.float32)
    bo = np.asarray(bo, dtype=np.float32)

    def wrep(W, g):
        # [C, 2, 96]: head h cols = W[64g+32h : 64g+32h+32, :].T tiled 3x
        out = np.empty((C, HPC, 96), np.float32)
        for h in range(HPC):
            blk = W[GD * g + HD * h : GD * g + HD * (h + 1), :].T  # [C, 32]
            out[:, h, :] = np.tile(blk, (1, 3))
        return out

    def b3(bvec, g, h):
        return np.tile(bvec[GD * g + HD * h : GD * g + HD * (h + 1)], 3)

    xf = x.reshape(B, C, N)
    in_maps = []
    for core in range(NC):
        b = core // 4
        g = core % 4
        sl = slice(GD * g, GD * (g + 1))
        w_all = np.concatenate(
            [
                wrep(Wq, g).reshape(C, 192),
                wrep(Wk, g).reshape(C, 192),
                Wv[sl, :].T,
            ],
            axis=1,
        )
        b4 = np.stack(
            [b3(bq, g, 0), b3(bq, g, 1), b3(bk, g, 0), b3(bk, g, 1)], axis=1
        )
        # bv 3 replicas (for batched V^T drains) then bo
        brep = np.concatenate(
            [np.tile(bv[sl], 3), bo if g == 0 else np.zeros(C, np.float32)]
        ).reshape(1, 3 * GD + C)
        in_maps.append(
            {
                "x": np.ascontiguousarray(xf[b].astype(NPBF16)),
                "w_all": np.ascontiguousarray(w_all.astype(NPBF16)),
                "b4": np.ascontiguousarray(b4),
                "brep": np.ascontiguousarray(brep),
                "wo_t": np.ascontiguousarray(Wo[:, sl].T.astype(NPBF16)),
            }
        )

    if _CACHED_NC is None:
        _CACHED_NC = build_nc()
    res = run_bass_kernel_spmd(_CACHED_NC, in_maps, core_ids=list(range(NC)))
    LAST_RESULTS = res

    out = np.zeros((B, C, N), dtype=np.float32)
    for core in range(NC):
        out[core // 4] += res.results[core]["out_t"].T
    return out.reshape(B, C, 48, 48)
